# revision 1
# baseline (speedup 1.0000x reference)
"""Trainium2 Bass kernel for BCNet-style bilinear head.

Computes logits[b,h,n,d] = sum_k hm[h,k] * v_[b,n,k] * q_[b,d,k] + h_bias
where v_ = v @ wn(Wv,gv).T + bv,  q_ = q @ wn(Wq,gq).T + bq,
wn(W,g) = (g/||W||_F) * W.

Restructured to minimize FLOPs (150 GF total instead of naive 219 GF):
  per batch b:
    q_T[k,d]    = (sq*Wq) @ q[b].T + bq           (contract C)
    qh[k,h*D+d] = hm[h,k] * q_T[k,d]
    GT[cv,hd]   = sum_k (sv*Wv)[k,cv] * qh[k,hd]  (contract K)
    bvrow[hd]   = sum_k bv[k]*hm[h,k]*q_T[k,d] (+h_bias)
    out[b][n,hd] = sum_cv vT[cv,n] * GT[cv,hd] + bvrow  (ones-row matmul pass)
Sharding: data-parallel over B=16 across 8 cores (2 batches/core).
All matmuls bf16 with fp32 PSUM accumulation (measured rel err ~3.5e-3).

Both batches' phase-1 runs inside the initial weight-DMA window; batch 1's
qh tiles reuse the wq SBUF slots (same shape) once wq is consumed.
"""

import sys

for _p in ("/opt/trn_rl_repo",):
    if _p not in sys.path:
        sys.path.insert(0, _p)

import numpy as np
import ml_dtypes

from concourse import bass, bacc, tile, mybir
from concourse.bass_utils import run_bass_kernel_spmd

BF16 = ml_dtypes.bfloat16
F32 = mybir.dt.float32
BF = mybir.dt.bfloat16
AF = mybir.ActivationFunctionType

B, N, C, D, K, H = 16, 1024, 1024, 128, 3072, 8
KT, CT, NT = K // 128, C // 128, N // 128  # 24, 8, 8
HD = H * D  # 1024
NCORES = 8
BPC = B // NCORES  # batches per core

_CACHE = {}


def _build_program(repeat=1):
    nc = bacc.Bacc("TRN2", target_bir_lowering=False, debug=False,
                   num_devices=NCORES)

    vT_d = nc.dram_tensor("vT", [BPC, C, N], BF, kind="ExternalInput")
    # qt2[p, ct*256 + b*128 + d] = q[b0+b, d, ct*128+p]  (batch pair packed)
    qT_d = nc.dram_tensor("qT", [128, CT * BPC * D], BF, kind="ExternalInput")
    # wq2[kt, p, ct*128+j] = (sq*Wq)[kt*128+j, ct*128+p]
    wqT_d = nc.dram_tensor("wqT", [KT, 128, CT * 128], BF, kind="ExternalInput")
    wv_d = nc.dram_tensor("wv", [K, C], BF, kind="ExternalInput")
    # bvhm[p, kt*8+h] = bv[k]*hm[h,k], k = kt*128+p
    bvhm_d = nc.dram_tensor("bvhm", [128, KT * H], BF, kind="ExternalInput")
    # cst columns: [0:KT] bq_t, [KT:KT+KT*H] hm_t, [216] h_bias col
    XBQ, XHM, XHB = 0, KT, KT + KT * H
    XTOT = KT + KT * H + 1
    cst_d = nc.dram_tensor("cst", [128, XTOT], F32, kind="ExternalInput")
    out_d = nc.dram_tensor("out", [BPC, H, N, D], F32, kind="ExternalOutput")

    with tile.TileContext(nc) as tc:
        with (
            tc.tile_pool(name="wq", bufs=1) as p_wq,
            tc.tile_pool(name="wv", bufs=1) as p_wv,
            tc.tile_pool(name="qh", bufs=1) as p_qh,
            tc.tile_pool(name="vt", bufs=1) as p_vt,
            tc.tile_pool(name="gt", bufs=1) as p_gt,
            tc.tile_pool(name="qsb", bufs=1) as p_qsb,
            tc.tile_pool(name="qt", bufs=1) as p_qt,
            tc.tile_pool(name="small", bufs=1) as p_small,
            tc.tile_pool(name="bvr", bufs=2) as p_bvr,
            tc.tile_pool(name="osb", bufs=4) as p_osb,
            tc.tile_pool(name="psq", bufs=2, space="PSUM") as ps_q,
            tc.tile_pool(name="psg", bufs=1, space="PSUM") as ps_g,
            tc.tile_pool(name="pso", bufs=2, space="PSUM") as ps_o,
        ):
          for _rep in range(repeat):
            cst_sb = p_small.tile([128, XTOT], F32)
            nc.sync.dma_start(cst_sb[:], cst_d.ap())
            bvhm_sb = p_small.tile([128, KT * H], BF)
            nc.sync.dma_start(bvhm_sb[:], bvhm_d.ap())

            QD = BPC * D  # both batches side by side in phase-1 rhs
            qt_sb = p_qt.tile([128, CT * QD], BF)
            nc.sync.dma_start(qt_sb[:], qT_d.ap())

            # wq per k-tile: phase-1 consumes slices as they stream in
            wq_sb = []
            for kt in range(KT):
                t = p_wq.tile([128, CT * 128], BF, tag=f"wq{kt}")
                nc.sync.dma_start(t[:], wqT_d.ap()[kt])
                wq_sb.append(t)

            # ---- phase 1 (both batches paired, N=256): q_T, qh ----
            qs = p_qsb.tile([128, KT * QD], BF)
            qh_tiles = [[], []]
            for kt in range(KT):
                pq = ps_q.tile([128, QD], F32)
                for ct in range(CT):
                    nc.tensor.matmul(
                        pq[:],
                        wq_sb[kt][:, ct * 128:(ct + 1) * 128],
                        qt_sb[:, ct * QD:(ct + 1) * QD],
                        start=(ct == 0), stop=(ct == CT - 1))
                nc.scalar.activation(
                    qs[:, kt * QD:(kt + 1) * QD], pq[:], AF.Identity,
                    bias=cst_sb[:, XBQ + kt:XBQ + kt + 1], scale=1.0)
                # qh[k, h*D:(h+1)*D] = hm[h,k] * q_T[k, :] — batch 0 inline
                # (feeds G(b0) immediately); batch 1 deferred below so the
                # DVE stream ahead of G(b0) is halved
                qh = p_qh.tile([128, HD], BF, tag=f"qh{kt}")
                qh_tiles[0].append(qh)
                for h in range(H):
                    nc.vector.tensor_scalar_mul(
                        qh[:, h * D:(h + 1) * D],
                        qs[:, kt * QD: kt * QD + D],
                        cst_sb[:, XHM + kt * H + h: XHM + kt * H + h + 1])
            for kt in range(KT):
                qh = p_wq.tile([128, HD], BF, tag=f"wq{kt}", name=f"qh1_{kt}")
                qh_tiles[1].append(qh)
                for h in range(H):
                    nc.vector.tensor_scalar_mul(
                        qh[:, h * D:(h + 1) * D],
                        qs[:, kt * QD + D: kt * QD + 2 * D],
                        cst_sb[:, XHM + kt * H + h: XHM + kt * H + h + 1])

            # bvterm[h,d] = sum_k bv[k]*hm[h,k]*q_T[k,d]; then to [1, HD] row
            bvrow = []
            for b in range(BPC):
                pbv = ps_q.tile([8, D], F32, tag="pq", name=f"pbv{b}")
                for kt in range(KT):
                    nc.tensor.matmul(
                        pbv[:],
                        bvhm_sb[:, kt * H:(kt + 1) * H],
                        qs[:, kt * QD + b * D: kt * QD + (b + 1) * D],
                        start=(kt == 0), stop=(kt == KT - 1))
                bvsb = p_bvr.tile([8, D], BF, tag="bvsb")
                nc.scalar.activation(bvsb[:], pbv[:], AF.Identity,
                                     bias=cst_sb[0:8, XHB:XHB + 1], scale=1.0)
                # broadcast to all 128 partitions: gather into row 0, then
                # log2 partition-doubling SBUF->SBUF DMAs (off critical path)
                big = p_bvr.tile([128, HD], BF, tag="bvrow")
                bvrow.append(big)
                for h in range(H):
                    nc.sync.dma_start(big[0:1, h * D:(h + 1) * D],
                                      bvsb[h:h + 1, :])
                p = 1
                while p < 128:
                    nc.sync.dma_start(big[p:2 * p, :], big[0:p, :])
                    p *= 2

            # ---- deferred big loads ----
            # wv as per-kt tiles so G's first pass can consume the stream
            wv_sb = []
            for kt in range(KT):
                t = p_wv.tile([128, C], BF, tag=f"wv{kt}")
                nc.sync.dma_start(t[:], wv_d.ap()[kt * 128:(kt + 1) * 128, :])
                wv_sb.append(t)

            for b in range(BPC):
                vt_sb = p_vt.tile([128, CT * N], BF)
                nc.sync.dma_start(
                    vt_sb[:].rearrange("p (ct n) -> p ct n", ct=CT),
                    vT_d.ap()[b].rearrange("(ct p) n -> p ct n", p=128))

                # ---- G: GT[cv, hd] = sum_k wv[k,cv] * qh[k,hd] ----
                # kt-outer with 4 concurrent PSUM groups: consumes wv slices
                # as they stream in instead of stalling on the full 6MB
                gt_sb = p_gt.tile([128, CT * HD], BF)
                for half in range(2):
                    for c2 in range(2):
                        cts = range(half * 4, half * 4 + 4)
                        pgs = {ct: ps_g.tile([128, 512], F32, tag=f"pg{ct % 4}",
                                             name=f"pg_{b}_{half}_{c2}_{ct}")
                               for ct in cts}
                        for kt in range(KT):
                            for ct in cts:
                                nc.tensor.matmul(
                                    pgs[ct][:],
                                    wv_sb[kt][:, ct * 128:(ct + 1) * 128],
                                    qh_tiles[b][kt][:, c2 * 512:(c2 + 1) * 512],
                                    start=(kt == 0), stop=(kt == KT - 1))
                        for i, ct in enumerate(cts):
                            # alternate engines so PSUM slots free in parallel
                            dst = gt_sb[:, ct * HD + c2 * 512:
                                        ct * HD + c2 * 512 + 512]
                            if i % 2 == 0:
                                nc.scalar.activation(dst, pgs[ct][:], AF.Copy)
                            else:
                                nc.vector.tensor_copy(dst, pgs[ct][:])

                # ---- final: out[n,hd] = sum_cv vT[cv,n]*GT[cv,hd] + bvrow ----
                for nt in range(NT):
                    for c2 in range(2):
                        po = ps_o.tile([128, 512], F32)
                        for ct in range(CT):
                            nc.tensor.matmul(
                                po[:],
                                vt_sb[:, ct * N + nt * 128: ct * N + (nt + 1) * 128],
                                gt_sb[:, ct * HD + c2 * 512: ct * HD + c2 * 512 + 512],
                                start=(ct == 0), stop=(ct == CT - 1))
                        ob = p_osb.tile([128, 512], F32)
                        nc.vector.tensor_tensor(
                            ob[:], po[:],
                            bvrow[b][:, c2 * 512:(c2 + 1) * 512],
                            mybir.AluOpType.add)
                        nc.sync.dma_start(
                            out_d.ap()[b, c2 * 4:(c2 + 1) * 4,
                                       nt * 128:(nt + 1) * 128, :]
                            .rearrange("h n d -> n h d"),
                            ob[:].rearrange("n (h d) -> n h d", h=4))

    nc.compile()
    return nc


def _get_program(repeat=1):
    key = f"nc{repeat}"
    if key not in _CACHE:
        _CACHE[key] = _build_program(repeat)
    return _CACHE[key]


def _prep_inputs(v, q, Wv, gv, bv, Wq, gq, bq, h_mat, h_bias):
    v = np.asarray(v, np.float32)
    q = np.asarray(q, np.float32)
    Wv = np.asarray(Wv, np.float32)
    Wq = np.asarray(Wq, np.float32)
    bv = np.asarray(bv, np.float32)
    bq = np.asarray(bq, np.float32)
    sv = np.float32(gv) / np.float32(np.linalg.norm(Wv))
    sq = np.float32(gq) / np.float32(np.linalg.norm(Wq))
    hm = np.asarray(h_mat, np.float32)[0, :, 0, :]  # (H, K)
    hb = np.asarray(h_bias, np.float32).reshape(H)

    wv_b = (Wv * sv).astype(BF16)                                     # (K, C)
    wqT_b = np.ascontiguousarray(
        (Wq * sq).reshape(KT, 128, CT, 128)
        .transpose(0, 3, 2, 1).reshape(KT, 128, CT * 128)).astype(BF16)
    vT = np.ascontiguousarray(v.transpose(0, 2, 1)).astype(BF16)      # (B,C,N)
    # qt2[core][p, ct*256 + b*128 + d] = q[b0+b, d, ct*128+p]
    qT = np.ascontiguousarray(
        q.reshape(NCORES, BPC, D, CT, 128).transpose(0, 4, 3, 1, 2)
        .reshape(NCORES, 128, CT * BPC * D)).astype(BF16)

    bq_t = np.ascontiguousarray(bq.reshape(KT, 128).T)                # (128,KT)
    hm_t = np.ascontiguousarray(
        hm.T.reshape(KT, 128, H).transpose(1, 0, 2).reshape(128, KT * H))
    hb_col = np.zeros((128, 1), np.float32)
    hb_col[:H, 0] = hb
    cst = np.concatenate([bq_t, hm_t, hb_col], axis=1).astype(np.float32)
    # bvhm[p, kt*8+h] = bv[k]*hm[h,k], k = kt*128+p
    bvhm = np.ascontiguousarray(
        (bv[None, :] * hm).T.reshape(KT, 128, H)
        .transpose(1, 0, 2).reshape(128, KT * H)).astype(BF16)

    in_maps = []
    for core in range(NCORES):
        b0 = core * BPC
        in_maps.append({
            "vT": np.ascontiguousarray(vT[b0:b0 + BPC]),
            "qT": qT[core],
            "wqT": wqT_b,
            "wv": wv_b,
            "bvhm": bvhm,
            "cst": cst,
        })
    return in_maps


def run_device(in_maps, **kw):
    nc = _get_program()
    return run_bass_kernel_spmd(nc, in_maps, list(range(NCORES)), **kw)


def kernel(v, q, Wv, gv, bv, Wq, gq, bq, h_mat, h_bias):
    in_maps = _prep_inputs(v, q, Wv, gv, bv, Wq, gq, bq, h_mat, h_bias)
    res = run_device(in_maps)
    out = np.empty((B, H, N, D), np.float32)
    for core in range(NCORES):
        b0 = core * BPC
        out[b0:b0 + BPC] = res.results[core]["out"]
    return out


if __name__ == "__main__":
    rng = np.random.default_rng(0)
    ins = {
        "v": rng.standard_normal((B, N, C), np.float32),
        "q": rng.standard_normal((B, D, C), np.float32),
        "Wv": rng.standard_normal((K, C), np.float32) * 0.02,
        "gv": np.ones((), np.float32),
        "bv": rng.standard_normal((K,), np.float32) * 0.02,
        "Wq": rng.standard_normal((K, C), np.float32) * 0.02,
        "gq": np.ones((), np.float32),
        "bq": rng.standard_normal((K,), np.float32) * 0.02,
        "h_mat": rng.standard_normal((1, H, 1, K), np.float32) * 0.02,
        "h_bias": np.zeros((1, H, 1, 1), np.float32),
    }
    out = kernel(**ins)
    print("out", out.shape, out.dtype, np.abs(out).max())



# revision 2
# speedup vs baseline: 7.8003x; 7.8003x over previous
"""Trainium2 Bass kernel for BCNet-style bilinear head.

Computes logits[b,h,n,d] = sum_k hm[h,k] * v_[b,n,k] * q_[b,d,k] + h_bias
where v_ = v @ wn(Wv,gv).T + bv,  q_ = q @ wn(Wq,gq).T + bq,
wn(W,g) = (g/||W||_F) * W.

Head-parallel M-route (120 GF total vs 150 GF for the GT-route):
expand the product; per head h (= per core):
  Mt[c',c]   = sum_k hm[h,k]*Wq'[k,c'] * Wv'[k,c]     (C x C, batch-indep)
  PT[c,bd]   = sum_c' Mt[c',c] * qT[c',bd] (+ u[c])   (u absorbs bq-term)
  out[b,n,d] = sum_c vT[b,c,n] * PT[c,b*D+d] + t3[b,d]
  t3[bd]     = sum_c' w[c'] * qT[c',bd] + t4          (bv-term + const)
with u[c] = sum_k hm*bq*Wv'[k,c], w[c'] = sum_k hm*bv*Wq'[k,c'],
t4 = sum_k hm*bv*bq + h_bias[h].
Sharding: head-parallel over H=8 across 8 cores; each core consumes the
full v/q (replicated) and emits out[:, h] — no collectives.
All matmuls bf16 with fp32 PSUM accumulation.

PSUM: one pool, 8 tags of [128,512] (16KB/part), reused by tag across
M / t3 / P / out phases. vT tiles ride a 48-slot ring over the retired
wqh/wv SBUF slots (M pass 2 walks kt in reverse so high-kt slots retire
first). P is software-pipelined one batch ahead of out to hide the
PSUM->SBUF copy latency.
"""

import sys

for _p in ("/opt/trn_rl_repo",):
    if _p not in sys.path:
        sys.path.insert(0, _p)

import numpy as np
import ml_dtypes

from concourse import bass, bacc, tile, mybir
from concourse.bass_utils import run_bass_kernel_spmd

BF16 = ml_dtypes.bfloat16
F32 = mybir.dt.float32
BF = mybir.dt.bfloat16
AF = mybir.ActivationFunctionType

B, N, C, D, K, H = 16, 1024, 1024, 128, 3072, 8
KT, CT, NT = K // 128, C // 128, N // 128  # 24, 8, 8
BD = B * D  # 2048
NCORES = 8
XU, XW, XT4 = 0, CT, 2 * CT  # cst columns: u tiles, w tiles, t4

_CACHE = {}


def _build_program(repeat=1):
    nc = bacc.Bacc("TRN2", target_bir_lowering=False, debug=False,
                   num_devices=NCORES)

    # wqh[kt,p,c'] = hm[h,k]*Wq'[k,c'], k = kt*128+p   (per-core, head h)
    wqh_d = nc.dram_tensor("wqh", [KT, 128, C], BF, kind="ExternalInput")
    wv_d = nc.dram_tensor("wv", [KT, 128, C], BF, kind="ExternalInput")
    # qT[ct,p,b*128+d] = q[b,d,ct*128+p]
    qT_d = nc.dram_tensor("qT", [CT, 128, BD], BF, kind="ExternalInput")
    # vT[b,ct,p,n] = v[b,n,ct*128+p]
    vT_d = nc.dram_tensor("vT", [B, CT, 128, N], BF, kind="ExternalInput")
    cst_d = nc.dram_tensor("cst", [128, 2 * CT + 1], F32, kind="ExternalInput")
    one_d = nc.dram_tensor("one", [128, 1], BF, kind="ExternalInput")
    out_d = nc.dram_tensor("out", [B, N, D], BF, kind="ExternalOutput")

    with tile.TileContext(nc) as tc:
        with (
            tc.tile_pool(name="wq", bufs=1) as p_wq,
            tc.tile_pool(name="wv", bufs=1) as p_wv,
            tc.tile_pool(name="qt", bufs=1) as p_qt,
            tc.tile_pool(name="mt", bufs=1) as p_mt,
            tc.tile_pool(name="pt", bufs=1) as p_pt,
            tc.tile_pool(name="t3", bufs=1) as p_t3,
            tc.tile_pool(name="small", bufs=1) as p_small,
            tc.tile_pool(name="ob", bufs=4) as p_ob,
            tc.tile_pool(name="ps", bufs=1, space="PSUM") as ps,
        ):
          for rep in range(repeat):
            R = f"r{rep}_"
            # ---- DMA: kt=0 weights first so M starts immediately ----
            wq_sb, wv_sb = [], []
            for kt in range(KT):
                tq = p_wq.tile([128, C], BF, tag=f"wq{kt}", name=f"{R}wq{kt}")
                nc.sync.dma_start(tq[:], wqh_d.ap()[kt])
                wq_sb.append(tq)
                tv = p_wv.tile([128, C], BF, tag=f"wv{kt}", name=f"{R}wv{kt}")
                nc.sync.dma_start(tv[:], wv_d.ap()[kt])
                wv_sb.append(tv)
                if kt == 0:
                    cst_sb = p_small.tile([128, 2 * CT + 1], F32, tag="cst",
                                          name=f"{R}cst")
                    nc.sync.dma_start(cst_sb[:], cst_d.ap())
                    one_sb = p_small.tile([128, 1], BF, tag="one",
                                          name=f"{R}one")
                    nc.sync.dma_start(one_sb[:], one_d.ap())
                    qt_sb = p_qt.tile([128, CT * BD], BF, tag="qt",
                                      name=f"{R}qt")
                    nc.sync.dma_start(
                        qt_sb[:].rearrange("p (ct j) -> p ct j", ct=CT),
                        qT_d.ap().rearrange("ct p j -> p ct j"))

            # ---- t3 partials on DVE (runs during M) ----
            ta = p_t3.tile([128, BD], BF, tag="ta", name=f"{R}ta")
            tb = p_t3.tile([128, BD], BF, tag="tb", name=f"{R}tb")
            nc.vector.tensor_scalar_mul(ta[:], qt_sb[:, 0:BD],
                                        cst_sb[:, XW:XW + 1])
            for ct in range(1, CT):
                nc.vector.tensor_scalar_mul(
                    tb[:], qt_sb[:, ct * BD:(ct + 1) * BD],
                    cst_sb[:, XW + ct:XW + ct + 1])
                nc.vector.tensor_tensor(ta[:], ta[:], tb[:],
                                        mybir.AluOpType.add)

            # ---- M: Mt[c',c] = sum_k wqh[k,c']*wv[k,c] ----
            # two c-half passes; pass 2 reversed so high-kt tiles retire
            # first (their SBUF slots become the vT ring, below)
            mt_sb = [p_mt.tile([128, C], BF, tag=f"mt{i}", name=f"{R}mt{i}")
                     for i in range(CT)]
            for half in range(2):
                kts = list(range(KT)) if half == 0 else \
                    list(range(KT - 1, -1, -1))
                pms = [ps.tile([128, 512], F32, tag=f"t{i}",
                               name=f"{R}pm{half}_{i}") for i in range(CT)]
                for kt in kts:
                    for i in range(CT):
                        nc.tensor.matmul(
                            pms[i][:],
                            wq_sb[kt][:, i * 128:(i + 1) * 128],
                            wv_sb[kt][:, half * 512:(half + 1) * 512],
                            start=(kt == kts[0]), stop=(kt == kts[-1]))
                for i in range(CT):
                    dst = mt_sb[i][:, half * 512:(half + 1) * 512]
                    if i % 2 == 0:
                        nc.scalar.activation(dst, pms[i][:], AF.Copy)
                    else:
                        nc.vector.tensor_copy(dst, pms[i][:])

            # ---- t3 row: partition-reduce + t4, broadcast to 128 rows ----
            t3bc = p_t3.tile([128, BD], BF, tag="t3bc", name=f"{R}t3bc")
            for j in range(4):
                pt3 = ps.tile([1, 512], F32, tag=f"t{j}", name=f"{R}t3ps{j}")
                nc.tensor.matmul(pt3[:], one_sb[:, 0:1],
                                 ta[:, j * 512:(j + 1) * 512],
                                 start=True, stop=True)
                nc.scalar.activation(t3bc[0:1, j * 512:(j + 1) * 512],
                                     pt3[:], AF.Identity,
                                     bias=cst_sb[0:1, XT4:XT4 + 1], scale=1.0)
            p = 1
            while p < 128:
                nc.sync.dma_start(t3bc[p:2 * p, :], t3bc[0:p, :])
                p *= 2

            # ---- per batch: P_b (pipelined one ahead) + out_{b-1} ----
            pt_sb = [p_pt.tile([128, BD], BF, tag=f"pt{i}", name=f"{R}pt{i}")
                     for i in range(CT)]
            vts = {}

            def load_vt(b):
                vts[b] = []
                for ct in range(CT):
                    g = (b * CT + ct) % (2 * KT)
                    skt = KT - 1 - g // 2
                    pool, tag = ((p_wq, f"wq{skt}") if g % 2 == 0
                                 else (p_wv, f"wv{skt}"))
                    t = pool.tile([128, C], BF, tag=tag, name=f"{R}vt{b}_{ct}")
                    nc.sync.dma_start(t[:], vT_d.ap()[b, ct])
                    vts[b].append(t)

            def p_phase(b):
                for ct in range(CT):
                    pp = ps.tile([128, 128], F32, tag=f"t{ct}",
                                 name=f"{R}pp{b}_{ct}")
                    for j in range(CT):
                        nc.tensor.matmul(
                            pp[:],
                            mt_sb[j][:, ct * 128:(ct + 1) * 128],
                            qt_sb[:, j * BD + b * 128:j * BD + (b + 1) * 128],
                            start=(j == 0), stop=(j == CT - 1))
                    nc.scalar.activation(
                        pt_sb[ct][:, b * 128:(b + 1) * 128], pp[:],
                        AF.Identity, bias=cst_sb[:, XU + ct:XU + ct + 1],
                        scale=1.0)

            def out_phase(b):
                for nt in range(NT):
                    po = ps.tile([128, 128], F32, tag=f"t{nt}",
                                 name=f"{R}po{b}_{nt}")
                    for ct in range(CT):
                        nc.tensor.matmul(
                            po[:],
                            vts[b][ct][:, nt * 128:(nt + 1) * 128],
                            pt_sb[ct][:, b * 128:(b + 1) * 128],
                            start=(ct == 0), stop=(ct == CT - 1))
                    ob = p_ob.tile([128, D], BF, tag=f"ob{nt % 4}",
                                   name=f"{R}ob{b}_{nt}")
                    nc.vector.tensor_tensor(
                        ob[:], po[:], t3bc[:, b * 128:(b + 1) * 128],
                        mybir.AluOpType.add)
                    nc.sync.dma_start(
                        out_d.ap()[b, nt * 128:(nt + 1) * 128, :], ob[:])
                del vts[b]

            load_vt(0)
            p_phase(0)
            for b in range(1, B):
                load_vt(b)
                p_phase(b)
                out_phase(b - 1)
            out_phase(B - 1)

    nc.compile()
    return nc


def _get_program(repeat=1):
    key = f"nc{repeat}"
    if key not in _CACHE:
        _CACHE[key] = _build_program(repeat)
    return _CACHE[key]


def _prep_inputs(v, q, Wv, gv, bv, Wq, gq, bq, h_mat, h_bias):
    v = np.asarray(v, np.float32)
    q = np.asarray(q, np.float32)
    Wv = np.asarray(Wv, np.float32)
    Wq = np.asarray(Wq, np.float32)
    bv = np.asarray(bv, np.float32)
    bq = np.asarray(bq, np.float32)
    sv = np.float32(gv) / np.float32(np.linalg.norm(Wv))
    sq = np.float32(gq) / np.float32(np.linalg.norm(Wq))
    hm = np.asarray(h_mat, np.float32)[0, :, 0, :]  # (H, K)
    hb = np.asarray(h_bias, np.float32).reshape(H)

    Wvp = Wv * sv  # (K, C)
    Wqp = Wq * sq
    wv_b = np.ascontiguousarray(Wvp.reshape(KT, 128, C)).astype(BF16)
    qT = np.ascontiguousarray(
        q.transpose(2, 0, 1).reshape(CT, 128, BD)).astype(BF16)
    vT = np.ascontiguousarray(
        v.transpose(0, 2, 1).reshape(B, CT, 128, N)).astype(BF16)
    one = np.ones((128, 1), BF16)

    in_maps = []
    for h in range(NCORES):
        wqh = np.ascontiguousarray(
            (hm[h][:, None] * Wqp).reshape(KT, 128, C)).astype(BF16)
        u = (hm[h] * bq) @ Wvp  # (C,)
        w = (hm[h] * bv) @ Wqp  # (C,)
        t4 = float((hm[h] * bv) @ bq) + float(hb[h])
        cst = np.zeros((128, 2 * CT + 1), np.float32)
        cst[:, XU:XU + CT] = u.reshape(CT, 128).T
        cst[:, XW:XW + CT] = w.reshape(CT, 128).T
        cst[0, XT4] = t4
        in_maps.append({
            "wqh": wqh,
            "wv": wv_b,
            "qT": qT,
            "vT": vT,
            "cst": cst,
            "one": one,
        })
    return in_maps


def run_device(in_maps, **kw):
    nc = _get_program()
    return run_bass_kernel_spmd(nc, in_maps, list(range(NCORES)), **kw)


def kernel(v, q, Wv, gv, bv, Wq, gq, bq, h_mat, h_bias):
    in_maps = _prep_inputs(v, q, Wv, gv, bv, Wq, gq, bq, h_mat, h_bias)
    res = run_device(in_maps)
    out = np.empty((B, H, N, D), np.float32)
    for h in range(NCORES):
        out[:, h] = res.results[h]["out"].astype(np.float32)
    return out


if __name__ == "__main__":
    rng = np.random.default_rng(0)
    ins = {
        "v": rng.standard_normal((B, N, C), np.float32),
        "q": rng.standard_normal((B, D, C), np.float32),
        "Wv": rng.standard_normal((K, C), np.float32) * 0.02,
        "gv": np.ones((), np.float32),
        "bv": rng.standard_normal((K,), np.float32) * 0.02,
        "Wq": rng.standard_normal((K, C), np.float32) * 0.02,
        "gq": np.ones((), np.float32),
        "bq": rng.standard_normal((K,), np.float32) * 0.02,
        "h_mat": rng.standard_normal((1, H, 1, K), np.float32) * 0.02,
        "h_bias": np.zeros((1, H, 1, 1), np.float32),
    }
    out = kernel(**ins)
    print("out", out.shape, out.dtype, np.abs(out).max())


# revision 10
# speedup vs baseline: 9.5274x; 1.2214x over previous
"""Trainium2 Bass kernel for BCNet-style bilinear head.

Computes logits[b,h,n,d] = sum_k hm[h,k] * v_[b,n,k] * q_[b,d,k] + h_bias
where v_ = v @ wn(Wv,gv).T + bv,  q_ = q @ wn(Wq,gq).T + bq,
wn(W,g) = (g/||W||_F) * W.

Head-parallel M-route (120 GF total vs 150 GF for the GT-route):
expand the product; per head h (= per core):
  Mt[c',c]   = sum_k hm[h,k]*Wq'[k,c'] * Wv'[k,c]     (C x C, batch-indep)
  PT[c,bd]   = sum_c' Mt[c',c] * qT[c',bd] (+ u[c])   (u absorbs bq-term)
  out[b,n,d] = sum_c vT[b,c,n] * PT[c,b*D+d] + t3[b,d]
  t3[bd]     = sum_c' w[c'] * qT[c',bd] + t4          (bv-term + const)
with u[c] = sum_k hm*bq*Wv'[k,c], w[c'] = sum_k hm*bv*Wq'[k,c'],
t4 = sum_k hm*bv*bq + h_bias[h].
Sharding: head-parallel over H=8 across 8 cores; each core consumes the
full v/q (replicated) and emits out[:, h] — no collectives.
All matmuls bf16 with fp32 PSUM accumulation.

PSUM: one pool, 8 tags of [128,512] (16KB/part), reused by tag across
M / t3 / P / out phases. vT tiles ride a 48-slot ring over the retired
wqh/wv SBUF slots (M pass 2 walks kt in reverse so high-kt slots retire
first). P is software-pipelined one batch ahead of out to hide the
PSUM->SBUF copy latency.
"""

import sys

for _p in ("/opt/trn_rl_repo",):
    if _p not in sys.path:
        sys.path.insert(0, _p)

import numpy as np
import ml_dtypes

from concourse import bass, bacc, tile, mybir
from concourse.bass_utils import run_bass_kernel_spmd

BF16 = ml_dtypes.bfloat16
F32 = mybir.dt.float32
BF = mybir.dt.bfloat16
AF = mybir.ActivationFunctionType

B, N, C, D, K, H = 16, 1024, 1024, 128, 3072, 8
KT, CT, NT = K // 128, C // 128, N // 128  # 24, 8, 8
BD = B * D  # 2048
NCORES = 8
XU, XW, XT4 = 0, CT, 2 * CT  # cst columns: u tiles, w tiles, t4

_CACHE = {}


def _build_program(repeat=1):
    nc = bacc.Bacc("TRN2", target_bir_lowering=False, debug=False,
                   num_devices=NCORES)

    # wqh[kt,p,c'] = hm[h,k]*Wq'[k,c'], k = kt*128+p   (per-core, head h)
    wqh_d = nc.dram_tensor("wqh", [KT, 128, C], BF, kind="ExternalInput")
    wv_d = nc.dram_tensor("wv", [KT, 128, C], BF, kind="ExternalInput")
    # qT[ct,p,b*128+d] = q[b,d,ct*128+p]
    qT_d = nc.dram_tensor("qT", [CT, 128, BD], BF, kind="ExternalInput")
    # vT[b,ct,p,n] = v[b,n,ct*128+p]
    vT_d = nc.dram_tensor("vT", [B, CT, 128, N], BF, kind="ExternalInput")
    cst_d = nc.dram_tensor("cst", [128, 2 * CT + 1], F32, kind="ExternalInput")
    one_d = nc.dram_tensor("one", [128, 1], BF, kind="ExternalInput")
    oner_d = nc.dram_tensor("oner", [1, 128], BF, kind="ExternalInput")
    out_d = nc.dram_tensor("out", [B, N, D], BF, kind="ExternalOutput")

    with tile.TileContext(nc) as tc:
        with (
            tc.tile_pool(name="wq", bufs=1) as p_wq,
            tc.tile_pool(name="wv", bufs=1) as p_wv,
            tc.tile_pool(name="qt", bufs=1) as p_qt,
            tc.tile_pool(name="mt", bufs=1) as p_mt,
            tc.tile_pool(name="pt", bufs=1) as p_pt,
            tc.tile_pool(name="t3", bufs=1) as p_t3,
            tc.tile_pool(name="small", bufs=1) as p_small,
            tc.tile_pool(name="ob", bufs=1) as p_ob,
            tc.tile_pool(name="ps", bufs=1, space="PSUM") as ps,
        ):
          for rep in range(repeat):
            R = f"r{rep}_"
            # ---- DMA: kt=0 weights first so M starts immediately ----
            wq_sb, wv_sb = [], []
            for kt in range(KT):
                tq = p_wq.tile([128, C], BF, tag=f"wq{kt}", name=f"{R}wq{kt}")
                nc.sync.dma_start(tq[:], wqh_d.ap()[kt])
                wq_sb.append(tq)
                tv = p_wv.tile([128, C], BF, tag=f"wv{kt}", name=f"{R}wv{kt}")
                nc.sync.dma_start(tv[:], wv_d.ap()[kt])
                wv_sb.append(tv)
                if kt == 0:
                    # small consts + qT ride the scalar engine's DGE queue so
                    # they never block the weight/vT stream on sync's queue
                    cst_sb = p_small.tile([128, 2 * CT + 1], F32, tag="cst",
                                          name=f"{R}cst")
                    nc.scalar.dma_start(cst_sb[:], cst_d.ap())
                    one_sb = p_small.tile([128, 1], BF, tag="one",
                                          name=f"{R}one")
                    nc.scalar.dma_start(one_sb[:], one_d.ap())
                    oner_sb = p_small.tile([1, 128], BF, tag="oner",
                                           name=f"{R}oner")
                    nc.scalar.dma_start(oner_sb[:], oner_d.ap())
                    qt_sb = p_qt.tile([128, CT * BD], BF, tag="qt",
                                      name=f"{R}qt")
                    for g in range(2):
                        nc.scalar.dma_start(
                            qt_sb[:, g * 4 * BD:(g + 1) * 4 * BD]
                            .rearrange("p (ct j) -> p ct j", ct=4),
                            qT_d.ap()[g * 4:(g + 1) * 4]
                            .rearrange("ct p j -> p ct j"))

            # ---- t3 partials on DVE (runs during M) ----
            ta = p_t3.tile([128, BD], BF, tag="ta", name=f"{R}ta")
            tb = p_t3.tile([128, BD], BF, tag="tb", name=f"{R}tb")
            nc.vector.tensor_scalar_mul(ta[:], qt_sb[:, 0:BD],
                                        cst_sb[:, XW:XW + 1])
            for ct in range(1, CT):
                nc.vector.tensor_scalar_mul(
                    tb[:], qt_sb[:, ct * BD:(ct + 1) * BD],
                    cst_sb[:, XW + ct:XW + ct + 1])
                nc.vector.tensor_tensor(ta[:], ta[:], tb[:],
                                        mybir.AluOpType.add)

            # ---- M: Mt[c',c] = sum_k wqh[k,c']*wv[k,c] ----
            # two c-half passes; pass 2 reversed so high-kt tiles retire
            # first (their SBUF slots become the vT ring, below)
            mt_sb = [p_mt.tile([128, C], BF, tag=f"mt{i}", name=f"{R}mt{i}")
                     for i in range(CT)]
            for half in range(2):
                kts = list(range(KT)) if half == 0 else \
                    list(range(KT - 1, -1, -1))
                pms = [ps.tile([128, 512], F32, tag=f"t{i}",
                               name=f"{R}pm{half}_{i}") for i in range(CT)]
                for kt in kts:
                    for i in range(CT):
                        nc.tensor.matmul(
                            pms[i][:],
                            wq_sb[kt][:, i * 128:(i + 1) * 128],
                            wv_sb[kt][:, half * 512:(half + 1) * 512],
                            start=(kt == kts[0]), stop=(kt == kts[-1]))
                for i in range(CT):
                    dst = mt_sb[i][:, half * 512:(half + 1) * 512]
                    if i % 2 == 0:
                        nc.scalar.activation(dst, pms[i][:], AF.Copy)
                    else:
                        nc.vector.tensor_copy(dst, pms[i][:])

            # ---- t3 row: partition-reduce + t4; broadcast via k=1 matmul ----
            t3row = p_t3.tile([1, BD], BF, tag="t3row", name=f"{R}t3row")
            t3bc = p_t3.tile([128, BD], BF, tag="t3bc", name=f"{R}t3bc")
            for j in range(4):
                pt3 = ps.tile([1, 512], F32, tag=f"t{j}", name=f"{R}t3ps{j}")
                nc.tensor.matmul(pt3[:], one_sb[:, 0:1],
                                 ta[:, j * 512:(j + 1) * 512],
                                 start=True, stop=True)
                nc.scalar.activation(t3row[0:1, j * 512:(j + 1) * 512],
                                     pt3[:], AF.Identity,
                                     bias=cst_sb[0:1, XT4:XT4 + 1], scale=1.0)
            for j in range(4):
                pb = ps.tile([128, 512], F32, tag=f"t{4 + j}",
                             name=f"{R}t3bc{j}")
                nc.tensor.matmul(pb[:], oner_sb[:],
                                 t3row[0:1, j * 512:(j + 1) * 512],
                                 start=True, stop=True)
                if j % 2 == 0:
                    nc.scalar.activation(t3bc[:, j * 512:(j + 1) * 512],
                                         pb[:], AF.Copy)
                else:
                    nc.vector.tensor_copy(t3bc[:, j * 512:(j + 1) * 512],
                                          pb[:])

            # ---- per batch: P_b (pipelined one ahead) + out_{b-1} ----
            pt_sb = [p_pt.tile([128, BD], BF, tag=f"pt{i}", name=f"{R}pt{i}")
                     for i in range(CT)]
            vts = {}

            def load_vt(b):
                vts[b] = []
                for ct in range(CT):
                    g = (b * CT + ct) % (2 * KT)
                    skt = KT - 1 - g // 2
                    pool, tag = ((p_wq, f"wq{skt}") if g % 2 == 0
                                 else (p_wv, f"wv{skt}"))
                    t = pool.tile([128, C], BF, tag=tag, name=f"{R}vt{b}_{ct}")
                    nc.sync.dma_start(t[:], vT_d.ap()[b, ct])
                    vts[b].append(t)

            def p_phase(b):
                for ct in range(CT):
                    pp = ps.tile([128, 128], F32, tag=f"t{ct}",
                                 name=f"{R}pp{b}_{ct}")
                    for j in range(CT):
                        nc.tensor.matmul(
                            pp[:],
                            mt_sb[j][:, ct * 128:(ct + 1) * 128],
                            qt_sb[:, j * BD + b * 128:j * BD + (b + 1) * 128],
                            start=(j == 0), stop=(j == CT - 1))
                    nc.scalar.activation(
                        pt_sb[ct][:, b * 128:(b + 1) * 128], pp[:],
                        AF.Identity, bias=cst_sb[:, XU + ct:XU + ct + 1],
                        scale=1.0)

            def out_phase(b):
                ob = p_ob.tile([128, NT * D], BF, tag=f"ob{b % 2}",
                               name=f"{R}ob{b}")
                for nt in range(NT):
                    po = ps.tile([128, 128], F32, tag=f"t{nt}",
                                 name=f"{R}po{b}_{nt}")
                    for ct in range(CT):
                        nc.tensor.matmul(
                            po[:],
                            vts[b][ct][:, nt * 128:(nt + 1) * 128],
                            pt_sb[ct][:, b * 128:(b + 1) * 128],
                            start=(ct == 0), stop=(ct == CT - 1))
                    nc.vector.tensor_tensor(
                        ob[:, nt * D:(nt + 1) * D], po[:],
                        t3bc[:, b * 128:(b + 1) * 128],
                        mybir.AluOpType.add)
                # one store per batch, on Activation's DGE queue (doesn't
                # block the sync-queue vT load stream)
                nc.scalar.dma_start(
                    out_d.ap()[b].rearrange("(nt p) d -> p nt d", p=128),
                    ob[:].rearrange("p (nt d) -> p nt d", nt=NT))
                del vts[b]

            load_vt(0)
            p_phase(0)
            for b in range(1, B):
                load_vt(b)
                p_phase(b)
                out_phase(b - 1)
            out_phase(B - 1)

    nc.compile()
    return nc


def _get_program(repeat=1):
    key = f"nc{repeat}"
    if key not in _CACHE:
        _CACHE[key] = _build_program(repeat)
    return _CACHE[key]


def _prep_inputs(v, q, Wv, gv, bv, Wq, gq, bq, h_mat, h_bias):
    v = np.asarray(v, np.float32)
    q = np.asarray(q, np.float32)
    Wv = np.asarray(Wv, np.float32)
    Wq = np.asarray(Wq, np.float32)
    bv = np.asarray(bv, np.float32)
    bq = np.asarray(bq, np.float32)
    sv = np.float32(gv) / np.float32(np.linalg.norm(Wv))
    sq = np.float32(gq) / np.float32(np.linalg.norm(Wq))
    hm = np.asarray(h_mat, np.float32)[0, :, 0, :]  # (H, K)
    hb = np.asarray(h_bias, np.float32).reshape(H)

    Wvp = Wv * sv  # (K, C)
    Wqp = Wq * sq
    wv_b = np.ascontiguousarray(Wvp.reshape(KT, 128, C)).astype(BF16)
    qT = np.ascontiguousarray(
        q.transpose(2, 0, 1).reshape(CT, 128, BD)).astype(BF16)
    vT = np.ascontiguousarray(
        v.transpose(0, 2, 1).reshape(B, CT, 128, N)).astype(BF16)
    one = np.ones((128, 1), BF16)
    oner = np.ones((1, 128), BF16)

    in_maps = []
    for h in range(NCORES):
        wqh = np.ascontiguousarray(
            (hm[h][:, None] * Wqp).reshape(KT, 128, C)).astype(BF16)
        u = (hm[h] * bq) @ Wvp  # (C,)
        w = (hm[h] * bv) @ Wqp  # (C,)
        t4 = float((hm[h] * bv) @ bq) + float(hb[h])
        cst = np.zeros((128, 2 * CT + 1), np.float32)
        cst[:, XU:XU + CT] = u.reshape(CT, 128).T
        cst[:, XW:XW + CT] = w.reshape(CT, 128).T
        cst[0, XT4] = t4
        in_maps.append({
            "wqh": wqh,
            "wv": wv_b,
            "qT": qT,
            "vT": vT,
            "cst": cst,
            "one": one,
            "oner": oner,
        })
    return in_maps


def run_device(in_maps, **kw):
    nc = _get_program()
    return run_bass_kernel_spmd(nc, in_maps, list(range(NCORES)), **kw)


def kernel(v, q, Wv, gv, bv, Wq, gq, bq, h_mat, h_bias):
    in_maps = _prep_inputs(v, q, Wv, gv, bv, Wq, gq, bq, h_mat, h_bias)
    res = run_device(in_maps)
    out = np.empty((B, H, N, D), np.float32)
    for h in range(NCORES):
        out[:, h] = res.results[h]["out"].astype(np.float32)
    return out


if __name__ == "__main__":
    rng = np.random.default_rng(0)
    ins = {
        "v": rng.standard_normal((B, N, C), np.float32),
        "q": rng.standard_normal((B, D, C), np.float32),
        "Wv": rng.standard_normal((K, C), np.float32) * 0.02,
        "gv": np.ones((), np.float32),
        "bv": rng.standard_normal((K,), np.float32) * 0.02,
        "Wq": rng.standard_normal((K, C), np.float32) * 0.02,
        "gq": np.ones((), np.float32),
        "bq": rng.standard_normal((K,), np.float32) * 0.02,
        "h_mat": rng.standard_normal((1, H, 1, K), np.float32) * 0.02,
        "h_bias": np.zeros((1, H, 1, 1), np.float32),
    }
    out = kernel(**ins)
    print("out", out.shape, out.dtype, np.abs(out).max())


# revision 11
# speedup vs baseline: 10.0579x; 1.0557x over previous
"""Trainium2 Bass kernel for BCNet-style bilinear head.

Computes logits[b,h,n,d] = sum_k hm[h,k] * v_[b,n,k] * q_[b,d,k] + h_bias
where v_ = v @ wn(Wv,gv).T + bv,  q_ = q @ wn(Wq,gq).T + bq,
wn(W,g) = (g/||W||_F) * W.

Head-parallel M-route (120 GF total vs 150 GF for the GT-route):
expand the product; per head h (= per core):
  Mt[c',c]   = sum_k hm[h,k]*Wq'[k,c'] * Wv'[k,c]     (C x C, batch-indep)
  PT[c,bd]   = sum_c' Mt[c',c] * qT[c',bd] (+ u[c])   (u absorbs bq-term)
  out[b,n,d] = sum_c vT[b,c,n] * PT[c,b*D+d] + t3[b,d]
  t3[bd]     = sum_c' w[c'] * qT[c',bd] + t4          (bv-term + const)
with u[c] = sum_k hm*bq*Wv'[k,c], w[c'] = sum_k hm*bv*Wq'[k,c'],
t4 = sum_k hm*bv*bq + h_bias[h].
Sharding: head-parallel over H=8 across 8 cores; each core consumes the
full v/q (replicated) and emits out[:, h] — no collectives.
All matmuls bf16 with fp32 PSUM accumulation.

PSUM: one pool, 8 tags of [128,512] (16KB/part), reused by tag across
M / t3 / P / out phases. vT tiles ride a 48-slot ring over the retired
wqh/wv SBUF slots (M pass 2 walks kt in reverse so high-kt slots retire
first). P is software-pipelined one batch ahead of out to hide the
PSUM->SBUF copy latency.
"""

import sys

for _p in ("/opt/trn_rl_repo",):
    if _p not in sys.path:
        sys.path.insert(0, _p)

import numpy as np
import ml_dtypes

from concourse import bass, bacc, tile, mybir
from concourse.bass_utils import run_bass_kernel_spmd

BF16 = ml_dtypes.bfloat16
F32 = mybir.dt.float32
BF = mybir.dt.bfloat16
AF = mybir.ActivationFunctionType

B, N, C, D, K, H = 16, 1024, 1024, 128, 3072, 8
KT, CT, NT = K // 128, C // 128, N // 128  # 24, 8, 8
BD = B * D  # 2048
NCORES = 8
XU, XW, XT4 = 0, CT, 2 * CT  # cst columns: u tiles, w tiles, t4

_CACHE = {}


def _build_program(repeat=1):
    nc = bacc.Bacc("TRN2", target_bir_lowering=False, debug=False,
                   num_devices=NCORES)

    # wqh[kt,p,c'] = hm[h,k]*Wq'[k,c'], k = kt*128+p   (per-core, head h)
    wqh_d = nc.dram_tensor("wqh", [KT, 128, C], BF, kind="ExternalInput")
    wv_d = nc.dram_tensor("wv", [KT, 128, C], BF, kind="ExternalInput")
    # qT[ct,p,b*128+d] = q[b,d,ct*128+p]
    qT_d = nc.dram_tensor("qT", [CT, 128, BD], BF, kind="ExternalInput")
    # vT[b,ct,p,n] = v[b,n,ct*128+p]
    vT_d = nc.dram_tensor("vT", [B, CT, 128, N], BF, kind="ExternalInput")
    cst_d = nc.dram_tensor("cst", [128, 2 * CT + 1], F32, kind="ExternalInput")
    one_d = nc.dram_tensor("one", [128, 1], BF, kind="ExternalInput")
    oner_d = nc.dram_tensor("oner", [1, 128], BF, kind="ExternalInput")
    out_d = nc.dram_tensor("out", [B, N, D], BF, kind="ExternalOutput")

    with tile.TileContext(nc) as tc:
        with (
            tc.tile_pool(name="wq", bufs=1) as p_wq,
            tc.tile_pool(name="wv", bufs=1) as p_wv,
            tc.tile_pool(name="qt", bufs=1) as p_qt,
            tc.tile_pool(name="mt", bufs=1) as p_mt,
            tc.tile_pool(name="pt", bufs=1) as p_pt,
            tc.tile_pool(name="t3", bufs=1) as p_t3,
            tc.tile_pool(name="small", bufs=1) as p_small,
            tc.tile_pool(name="ob", bufs=1) as p_ob,
            tc.tile_pool(name="ps", bufs=1, space="PSUM") as ps,
        ):
          for rep in range(repeat):
            R = f"r{rep}_"
            # ---- DMA: kt=0 weights first so M starts immediately ----
            wq_sb, wv_sb = [], []
            for kt in range(KT):
                tq = p_wq.tile([128, C], BF, tag=f"wq{kt}", name=f"{R}wq{kt}")
                nc.sync.dma_start(tq[:], wqh_d.ap()[kt])
                wq_sb.append(tq)
                tv = p_wv.tile([128, C], BF, tag=f"wv{kt}", name=f"{R}wv{kt}")
                nc.sync.dma_start(tv[:], wv_d.ap()[kt])
                wv_sb.append(tv)
                if kt == 0:
                    # small consts ride the scalar engine's DGE queue
                    cst_sb = p_small.tile([128, 2 * CT + 1], F32, tag="cst",
                                          name=f"{R}cst")
                    nc.scalar.dma_start(cst_sb[:], cst_d.ap())
                    one_sb = p_small.tile([128, 1], BF, tag="one",
                                          name=f"{R}one")
                    nc.scalar.dma_start(one_sb[:], one_d.ap())
                    oner_sb = p_small.tile([1, 128], BF, tag="oner",
                                           name=f"{R}oner")
                    nc.scalar.dma_start(oner_sb[:], oner_d.ap())
                    qt_sb = p_qt.tile([128, CT * BD], BF, tag="qt",
                                      name=f"{R}qt")
                # qT streams on the sync queue, interleaved between weight
                # tiles from kt>=6 so the PE's early M tiles aren't starved
                if kt >= 6 and kt % 2 == 0 and kt < 6 + 2 * CT:
                    g = (kt - 6) // 2
                    nc.sync.dma_start(
                        qt_sb[:, g * BD:(g + 1) * BD], qT_d.ap()[g])

            # ---- t3 partials on DVE (runs during M) ----
            ta = p_t3.tile([128, BD], BF, tag="ta", name=f"{R}ta")
            tb = p_t3.tile([128, BD], BF, tag="tb", name=f"{R}tb")
            nc.vector.tensor_scalar_mul(ta[:], qt_sb[:, 0:BD],
                                        cst_sb[:, XW:XW + 1])
            for ct in range(1, CT):
                nc.vector.tensor_scalar_mul(
                    tb[:], qt_sb[:, ct * BD:(ct + 1) * BD],
                    cst_sb[:, XW + ct:XW + ct + 1])
                nc.vector.tensor_tensor(ta[:], ta[:], tb[:],
                                        mybir.AluOpType.add)

            # ---- M: Mt[c',c] = sum_k wqh[k,c']*wv[k,c] ----
            # two c-half passes; pass 2 reversed so high-kt tiles retire
            # first (their SBUF slots become the vT ring, below)
            mt_sb = [p_mt.tile([128, C], BF, tag=f"mt{i}", name=f"{R}mt{i}")
                     for i in range(CT)]
            for half in range(2):
                kts = list(range(KT)) if half == 0 else \
                    list(range(KT - 1, -1, -1))
                pms = [ps.tile([128, 512], F32, tag=f"t{i}",
                               name=f"{R}pm{half}_{i}") for i in range(CT)]
                for kt in kts:
                    for i in range(CT):
                        nc.tensor.matmul(
                            pms[i][:],
                            wq_sb[kt][:, i * 128:(i + 1) * 128],
                            wv_sb[kt][:, half * 512:(half + 1) * 512],
                            start=(kt == kts[0]), stop=(kt == kts[-1]))
                for i in range(CT):
                    dst = mt_sb[i][:, half * 512:(half + 1) * 512]
                    if i % 2 == 0:
                        nc.scalar.activation(dst, pms[i][:], AF.Copy)
                    else:
                        nc.vector.tensor_copy(dst, pms[i][:])

            # ---- t3 row: partition-reduce + t4; broadcast via k=1 matmul ----
            t3row = p_t3.tile([1, BD], BF, tag="t3row", name=f"{R}t3row")
            t3bc = p_t3.tile([128, BD], BF, tag="t3bc", name=f"{R}t3bc")
            for j in range(4):
                pt3 = ps.tile([1, 512], F32, tag=f"t{j}", name=f"{R}t3ps{j}")
                nc.tensor.matmul(pt3[:], one_sb[:, 0:1],
                                 ta[:, j * 512:(j + 1) * 512],
                                 start=True, stop=True)
                nc.scalar.activation(t3row[0:1, j * 512:(j + 1) * 512],
                                     pt3[:], AF.Identity,
                                     bias=cst_sb[0:1, XT4:XT4 + 1], scale=1.0)
            for j in range(4):
                pb = ps.tile([128, 512], F32, tag=f"t{4 + j}",
                             name=f"{R}t3bc{j}")
                nc.tensor.matmul(pb[:], oner_sb[:],
                                 t3row[0:1, j * 512:(j + 1) * 512],
                                 start=True, stop=True)
                if j % 2 == 0:
                    nc.scalar.activation(t3bc[:, j * 512:(j + 1) * 512],
                                         pb[:], AF.Copy)
                else:
                    nc.vector.tensor_copy(t3bc[:, j * 512:(j + 1) * 512],
                                          pb[:])

            # ---- per batch: P_b (pipelined one ahead) + out_{b-1} ----
            pt_sb = [p_pt.tile([128, BD], BF, tag=f"pt{i}", name=f"{R}pt{i}")
                     for i in range(CT)]
            vts = {}

            def load_vt(b):
                vts[b] = []
                for ct in range(CT):
                    g = (b * CT + ct) % (2 * KT)
                    skt = KT - 1 - g // 2
                    pool, tag = ((p_wq, f"wq{skt}") if g % 2 == 0
                                 else (p_wv, f"wv{skt}"))
                    t = pool.tile([128, C], BF, tag=tag, name=f"{R}vt{b}_{ct}")
                    nc.sync.dma_start(t[:], vT_d.ap()[b, ct])
                    vts[b].append(t)

            def p_phase(b):
                for ct in range(CT):
                    pp = ps.tile([128, 128], F32, tag=f"t{ct}",
                                 name=f"{R}pp{b}_{ct}")
                    for j in range(CT):
                        nc.tensor.matmul(
                            pp[:],
                            mt_sb[j][:, ct * 128:(ct + 1) * 128],
                            qt_sb[:, j * BD + b * 128:j * BD + (b + 1) * 128],
                            start=(j == 0), stop=(j == CT - 1))
                    nc.scalar.activation(
                        pt_sb[ct][:, b * 128:(b + 1) * 128], pp[:],
                        AF.Identity, bias=cst_sb[:, XU + ct:XU + ct + 1],
                        scale=1.0)

            def out_phase(b):
                ob = p_ob.tile([128, NT * D], BF, tag=f"ob{b % 2}",
                               name=f"{R}ob{b}")
                for nt in range(NT):
                    po = ps.tile([128, 128], F32, tag=f"t{nt}",
                                 name=f"{R}po{b}_{nt}")
                    for ct in range(CT):
                        nc.tensor.matmul(
                            po[:],
                            vts[b][ct][:, nt * 128:(nt + 1) * 128],
                            pt_sb[ct][:, b * 128:(b + 1) * 128],
                            start=(ct == 0), stop=(ct == CT - 1))
                    nc.vector.tensor_tensor(
                        ob[:, nt * D:(nt + 1) * D], po[:],
                        t3bc[:, b * 128:(b + 1) * 128],
                        mybir.AluOpType.add)
                # one store per batch, on Activation's DGE queue (doesn't
                # block the sync-queue vT load stream)
                nc.scalar.dma_start(
                    out_d.ap()[b].rearrange("(nt p) d -> p nt d", p=128),
                    ob[:].rearrange("p (nt d) -> p nt d", nt=NT))
                del vts[b]

            load_vt(0)
            p_phase(0)
            for b in range(1, B):
                load_vt(b)
                p_phase(b)
                out_phase(b - 1)
            out_phase(B - 1)

    nc.compile()
    return nc


def _get_program(repeat=1):
    key = f"nc{repeat}"
    if key not in _CACHE:
        _CACHE[key] = _build_program(repeat)
    return _CACHE[key]


def _prep_inputs(v, q, Wv, gv, bv, Wq, gq, bq, h_mat, h_bias):
    v = np.asarray(v, np.float32)
    q = np.asarray(q, np.float32)
    Wv = np.asarray(Wv, np.float32)
    Wq = np.asarray(Wq, np.float32)
    bv = np.asarray(bv, np.float32)
    bq = np.asarray(bq, np.float32)
    sv = np.float32(gv) / np.float32(np.linalg.norm(Wv))
    sq = np.float32(gq) / np.float32(np.linalg.norm(Wq))
    hm = np.asarray(h_mat, np.float32)[0, :, 0, :]  # (H, K)
    hb = np.asarray(h_bias, np.float32).reshape(H)

    Wvp = Wv * sv  # (K, C)
    Wqp = Wq * sq
    wv_b = np.ascontiguousarray(Wvp.reshape(KT, 128, C)).astype(BF16)
    qT = np.ascontiguousarray(
        q.transpose(2, 0, 1).reshape(CT, 128, BD)).astype(BF16)
    vT = np.ascontiguousarray(
        v.transpose(0, 2, 1).reshape(B, CT, 128, N)).astype(BF16)
    one = np.ones((128, 1), BF16)
    oner = np.ones((1, 128), BF16)

    in_maps = []
    for h in range(NCORES):
        wqh = np.ascontiguousarray(
            (hm[h][:, None] * Wqp).reshape(KT, 128, C)).astype(BF16)
        u = (hm[h] * bq) @ Wvp  # (C,)
        w = (hm[h] * bv) @ Wqp  # (C,)
        t4 = float((hm[h] * bv) @ bq) + float(hb[h])
        cst = np.zeros((128, 2 * CT + 1), np.float32)
        cst[:, XU:XU + CT] = u.reshape(CT, 128).T
        cst[:, XW:XW + CT] = w.reshape(CT, 128).T
        cst[0, XT4] = t4
        in_maps.append({
            "wqh": wqh,
            "wv": wv_b,
            "qT": qT,
            "vT": vT,
            "cst": cst,
            "one": one,
            "oner": oner,
        })
    return in_maps


def run_device(in_maps, **kw):
    nc = _get_program()
    return run_bass_kernel_spmd(nc, in_maps, list(range(NCORES)), **kw)


def kernel(v, q, Wv, gv, bv, Wq, gq, bq, h_mat, h_bias):
    in_maps = _prep_inputs(v, q, Wv, gv, bv, Wq, gq, bq, h_mat, h_bias)
    res = run_device(in_maps)
    out = np.empty((B, H, N, D), np.float32)
    for h in range(NCORES):
        out[:, h] = res.results[h]["out"].astype(np.float32)
    return out


if __name__ == "__main__":
    rng = np.random.default_rng(0)
    ins = {
        "v": rng.standard_normal((B, N, C), np.float32),
        "q": rng.standard_normal((B, D, C), np.float32),
        "Wv": rng.standard_normal((K, C), np.float32) * 0.02,
        "gv": np.ones((), np.float32),
        "bv": rng.standard_normal((K,), np.float32) * 0.02,
        "Wq": rng.standard_normal((K, C), np.float32) * 0.02,
        "gq": np.ones((), np.float32),
        "bq": rng.standard_normal((K,), np.float32) * 0.02,
        "h_mat": rng.standard_normal((1, H, 1, K), np.float32) * 0.02,
        "h_bias": np.zeros((1, H, 1, 1), np.float32),
    }
    out = kernel(**ins)
    print("out", out.shape, out.dtype, np.abs(out).max())


# revision 14
# speedup vs baseline: 10.3283x; 1.0269x over previous
"""Trainium2 Bass kernel for BCNet-style bilinear head.

Computes logits[b,h,n,d] = sum_k hm[h,k] * v_[b,n,k] * q_[b,d,k] + h_bias
where v_ = v @ wn(Wv,gv).T + bv,  q_ = q @ wn(Wq,gq).T + bq,
wn(W,g) = (g/||W||_F) * W.

Head-parallel M-route (120 GF total vs 150 GF for the GT-route):
expand the product; per head h (= per core):
  Mt[c',c]   = sum_k hm[h,k]*Wq'[k,c'] * Wv'[k,c]     (C x C, batch-indep)
  PT[c,bd]   = sum_c' Mt[c',c] * qT[c',bd] (+ u[c])   (u absorbs bq-term)
  out[b,n,d] = sum_c vT[b,c,n] * PT[c,b*D+d] + t3[b,d]
  t3[bd]     = sum_c' w[c'] * qT[c',bd] + t4          (bv-term + const)
with u[c] = sum_k hm*bq*Wv'[k,c], w[c'] = sum_k hm*bv*Wq'[k,c'],
t4 = sum_k hm*bv*bq + h_bias[h].
Sharding: head-parallel over H=8 across 8 cores; each core consumes the
full v/q (replicated) and emits out[:, h] — no collectives.
All matmuls bf16 with fp32 PSUM accumulation.

PSUM: one pool, 8 tags of [128,512] (16KB/part), reused by tag across
M / t3 / P / out phases. vT tiles ride a 48-slot ring over the retired
wqh/wv SBUF slots (M pass 2 walks kt in reverse so high-kt slots retire
first). P is software-pipelined one batch ahead of out to hide the
PSUM->SBUF copy latency.
"""

import sys

for _p in ("/opt/trn_rl_repo",):
    if _p not in sys.path:
        sys.path.insert(0, _p)

import numpy as np
import ml_dtypes

from concourse import bass, bacc, tile, mybir
from concourse.bass_utils import run_bass_kernel_spmd

BF16 = ml_dtypes.bfloat16
F32 = mybir.dt.float32
BF = mybir.dt.bfloat16
AF = mybir.ActivationFunctionType

B, N, C, D, K, H = 16, 1024, 1024, 128, 3072, 8
KT, CT, NT = K // 128, C // 128, N // 128  # 24, 8, 8
BD = B * D  # 2048
NCORES = 8
XU, XW, XT4 = 0, CT, 2 * CT  # cst columns: u tiles, w tiles, t4

_CACHE = {}


def _build_program(repeat=1):
    nc = bacc.Bacc("TRN2", target_bir_lowering=False, debug=False,
                   num_devices=NCORES)

    # wqh[kt,p,c'] = hm[h,k]*Wq'[k,c'], k = kt*128+p   (per-core, head h)
    wqh_d = nc.dram_tensor("wqh", [KT, 128, C], BF, kind="ExternalInput")
    wv_d = nc.dram_tensor("wv", [KT, 128, C], BF, kind="ExternalInput")
    # qT[ct,p,b*128+d] = q[b,d,ct*128+p]
    qT_d = nc.dram_tensor("qT", [CT, 128, BD], BF, kind="ExternalInput")
    # vT[b,ct,p,n] = v[b,n,ct*128+p]
    vT_d = nc.dram_tensor("vT", [B, CT, 128, N], BF, kind="ExternalInput")
    cst_d = nc.dram_tensor("cst", [128, 2 * CT + 1], F32, kind="ExternalInput")
    one_d = nc.dram_tensor("one", [128, 1], BF, kind="ExternalInput")
    oner_d = nc.dram_tensor("oner", [1, 128], BF, kind="ExternalInput")
    out_d = nc.dram_tensor("out", [B, N, D], BF, kind="ExternalOutput")

    with tile.TileContext(nc) as tc:
        with (
            tc.tile_pool(name="wq", bufs=1) as p_wq,
            tc.tile_pool(name="wv", bufs=1) as p_wv,
            tc.tile_pool(name="qt", bufs=1) as p_qt,
            tc.tile_pool(name="mt", bufs=1) as p_mt,
            tc.tile_pool(name="pt", bufs=1) as p_pt,
            tc.tile_pool(name="t3", bufs=1) as p_t3,
            tc.tile_pool(name="small", bufs=1) as p_small,
            tc.tile_pool(name="ob", bufs=1) as p_ob,
            tc.tile_pool(name="ps", bufs=1, space="PSUM") as ps,
        ):
          for rep in range(repeat):
            R = f"r{rep}_"
            # ---- DMA: kt=0 weights first so M starts immediately ----
            wq_sb, wv_sb = [], []
            for kt in range(KT):
                tq = p_wq.tile([128, C], BF, tag=f"wq{kt}", name=f"{R}wq{kt}")
                nc.sync.dma_start(tq[:], wqh_d.ap()[kt])
                wq_sb.append(tq)
                tv = p_wv.tile([128, C], BF, tag=f"wv{kt}", name=f"{R}wv{kt}")
                nc.sync.dma_start(tv[:], wv_d.ap()[kt])
                wv_sb.append(tv)
                if kt == 0:
                    # small consts ride the scalar engine's DGE queue
                    cst_sb = p_small.tile([128, 2 * CT + 1], F32, tag="cst",
                                          name=f"{R}cst")
                    nc.scalar.dma_start(cst_sb[:], cst_d.ap())
                    one_sb = p_small.tile([128, 1], BF, tag="one",
                                          name=f"{R}one")
                    nc.scalar.dma_start(one_sb[:], one_d.ap())
                    oner_sb = p_small.tile([1, 128], BF, tag="oner",
                                           name=f"{R}oner")
                    nc.scalar.dma_start(oner_sb[:], oner_d.ap())
                    qt_sb = p_qt.tile([128, CT * BD], BF, tag="qt",
                                      name=f"{R}qt")


            # qT streams on the sync queue after all weight tiles: M pass 1
            # consumes the weight stream at DMA rate, while pass 2 re-reads
            # SBUF — qT (and then vT) ride the pass-2 bus-idle window
            for g in range(CT):
                nc.sync.dma_start(qt_sb[:, g * BD:(g + 1) * BD], qT_d.ap()[g])

            # ---- t3 partials on DVE (runs during M) ----
            ta = p_t3.tile([128, BD], BF, tag="ta", name=f"{R}ta")
            tb = p_t3.tile([128, BD], BF, tag="tb", name=f"{R}tb")
            nc.vector.tensor_scalar_mul(ta[:], qt_sb[:, 0:BD],
                                        cst_sb[:, XW:XW + 1])
            for ct in range(1, CT):
                nc.vector.tensor_scalar_mul(
                    tb[:], qt_sb[:, ct * BD:(ct + 1) * BD],
                    cst_sb[:, XW + ct:XW + ct + 1])
                nc.vector.tensor_tensor(ta[:], ta[:], tb[:],
                                        mybir.AluOpType.add)

            # ---- M: Mt[c',c] = sum_k wqh[k,c']*wv[k,c] ----
            # two c-half passes; pass 2 reversed so high-kt tiles retire
            # first (their SBUF slots become the vT ring, below)
            mt_sb = [p_mt.tile([128, C], BF, tag=f"mt{i}", name=f"{R}mt{i}")
                     for i in range(CT)]
            for half in range(2):
                kts = list(range(KT)) if half == 0 else \
                    list(range(KT - 1, -1, -1))
                pms = [ps.tile([128, 512], F32, tag=f"t{i}",
                               name=f"{R}pm{half}_{i}") for i in range(CT)]
                for kt in kts:
                    for i in range(CT):
                        nc.tensor.matmul(
                            pms[i][:],
                            wq_sb[kt][:, i * 128:(i + 1) * 128],
                            wv_sb[kt][:, half * 512:(half + 1) * 512],
                            start=(kt == kts[0]), stop=(kt == kts[-1]))
                for i in range(CT):
                    dst = mt_sb[i][:, half * 512:(half + 1) * 512]
                    if i % 2 == 0:
                        nc.scalar.activation(dst, pms[i][:], AF.Copy)
                    else:
                        nc.vector.tensor_copy(dst, pms[i][:])

            # ---- t3 row: partition-reduce + t4; broadcast via k=1 matmul ----
            t3row = p_t3.tile([1, BD], BF, tag="t3row", name=f"{R}t3row")
            t3bc = p_t3.tile([128, BD], BF, tag="t3bc", name=f"{R}t3bc")
            for j in range(4):
                pt3 = ps.tile([1, 512], F32, tag=f"t{j}", name=f"{R}t3ps{j}")
                nc.tensor.matmul(pt3[:], one_sb[:, 0:1],
                                 ta[:, j * 512:(j + 1) * 512],
                                 start=True, stop=True)
                nc.scalar.activation(t3row[0:1, j * 512:(j + 1) * 512],
                                     pt3[:], AF.Identity,
                                     bias=cst_sb[0:1, XT4:XT4 + 1], scale=1.0)
            for j in range(4):
                pb = ps.tile([128, 512], F32, tag=f"t{4 + j}",
                             name=f"{R}t3bc{j}")
                nc.tensor.matmul(pb[:], oner_sb[:],
                                 t3row[0:1, j * 512:(j + 1) * 512],
                                 start=True, stop=True)
                if j % 2 == 0:
                    nc.scalar.activation(t3bc[:, j * 512:(j + 1) * 512],
                                         pb[:], AF.Copy)
                else:
                    nc.vector.tensor_copy(t3bc[:, j * 512:(j + 1) * 512],
                                          pb[:])

            # ---- per batch: P_b (pipelined one ahead) + out_{b-1} ----
            pt_sb = [p_pt.tile([128, BD], BF, tag=f"pt{i}", name=f"{R}pt{i}")
                     for i in range(CT)]
            vts = {}

            def load_vt(b):
                vts[b] = []
                for ct in range(CT):
                    g = (b * CT + ct) % (2 * KT)
                    skt = KT - 1 - g // 2
                    pool, tag = ((p_wq, f"wq{skt}") if g % 2 == 0
                                 else (p_wv, f"wv{skt}"))
                    t = pool.tile([128, C], BF, tag=tag, name=f"{R}vt{b}_{ct}")
                    nc.sync.dma_start(t[:], vT_d.ap()[b, ct])
                    vts[b].append(t)

            def p_phase(b):
                for ct in range(CT):
                    pp = ps.tile([128, 128], F32, tag=f"t{ct}",
                                 name=f"{R}pp{b}_{ct}")
                    for j in range(CT):
                        nc.tensor.matmul(
                            pp[:],
                            mt_sb[j][:, ct * 128:(ct + 1) * 128],
                            qt_sb[:, j * BD + b * 128:j * BD + (b + 1) * 128],
                            start=(j == 0), stop=(j == CT - 1))
                    nc.scalar.activation(
                        pt_sb[ct][:, b * 128:(b + 1) * 128], pp[:],
                        AF.Identity, bias=cst_sb[:, XU + ct:XU + ct + 1],
                        scale=1.0)

            def out_phase(b):
                ob = p_ob.tile([128, NT * D], BF, tag=f"ob{b % 2}",
                               name=f"{R}ob{b}")
                for nt in range(NT):
                    po = ps.tile([128, 128], F32, tag=f"t{nt}",
                                 name=f"{R}po{b}_{nt}")
                    for ct in range(CT):
                        nc.tensor.matmul(
                            po[:],
                            vts[b][ct][:, nt * 128:(nt + 1) * 128],
                            pt_sb[ct][:, b * 128:(b + 1) * 128],
                            start=(ct == 0), stop=(ct == CT - 1))
                    nc.vector.tensor_tensor(
                        ob[:, nt * D:(nt + 1) * D], po[:],
                        t3bc[:, b * 128:(b + 1) * 128],
                        mybir.AluOpType.add)
                # one store per batch, on Activation's DGE queue (doesn't
                # block the sync-queue vT load stream); the final batch is
                # split in two so the first half overlaps the last adds
                if b == B - 1:
                    for g in range(2):
                        nc.scalar.dma_start(
                            out_d.ap()[b, g * 512:(g + 1) * 512, :]
                            .rearrange("(nt p) d -> p nt d", p=128),
                            ob[:, g * 4 * D:(g + 1) * 4 * D]
                            .rearrange("p (nt d) -> p nt d", nt=4))
                else:
                    nc.scalar.dma_start(
                        out_d.ap()[b].rearrange("(nt p) d -> p nt d", p=128),
                        ob[:].rearrange("p (nt d) -> p nt d", nt=NT))
                del vts[b]

            load_vt(0)
            p_phase(0)
            for b in range(1, B):
                load_vt(b)
                p_phase(b)
                out_phase(b - 1)
            out_phase(B - 1)

    nc.compile()
    return nc


def _get_program(repeat=1):
    key = f"nc{repeat}"
    if key not in _CACHE:
        _CACHE[key] = _build_program(repeat)
    return _CACHE[key]


def _prep_inputs(v, q, Wv, gv, bv, Wq, gq, bq, h_mat, h_bias):
    v = np.asarray(v, np.float32)
    q = np.asarray(q, np.float32)
    Wv = np.asarray(Wv, np.float32)
    Wq = np.asarray(Wq, np.float32)
    bv = np.asarray(bv, np.float32)
    bq = np.asarray(bq, np.float32)
    sv = np.float32(gv) / np.float32(np.linalg.norm(Wv))
    sq = np.float32(gq) / np.float32(np.linalg.norm(Wq))
    hm = np.asarray(h_mat, np.float32)[0, :, 0, :]  # (H, K)
    hb = np.asarray(h_bias, np.float32).reshape(H)

    Wvp = Wv * sv  # (K, C)
    Wqp = Wq * sq
    wv_b = np.ascontiguousarray(Wvp.reshape(KT, 128, C)).astype(BF16)
    qT = np.ascontiguousarray(
        q.transpose(2, 0, 1).reshape(CT, 128, BD)).astype(BF16)
    vT = np.ascontiguousarray(
        v.transpose(0, 2, 1).reshape(B, CT, 128, N)).astype(BF16)
    one = np.ones((128, 1), BF16)
    oner = np.ones((1, 128), BF16)

    in_maps = []
    for h in range(NCORES):
        wqh = np.ascontiguousarray(
            (hm[h][:, None] * Wqp).reshape(KT, 128, C)).astype(BF16)
        u = (hm[h] * bq) @ Wvp  # (C,)
        w = (hm[h] * bv) @ Wqp  # (C,)
        t4 = float((hm[h] * bv) @ bq) + float(hb[h])
        cst = np.zeros((128, 2 * CT + 1), np.float32)
        cst[:, XU:XU + CT] = u.reshape(CT, 128).T
        cst[:, XW:XW + CT] = w.reshape(CT, 128).T
        cst[0, XT4] = t4
        in_maps.append({
            "wqh": wqh,
            "wv": wv_b,
            "qT": qT,
            "vT": vT,
            "cst": cst,
            "one": one,
            "oner": oner,
        })
    return in_maps


def run_device(in_maps, **kw):
    nc = _get_program()
    return run_bass_kernel_spmd(nc, in_maps, list(range(NCORES)), **kw)


def kernel(v, q, Wv, gv, bv, Wq, gq, bq, h_mat, h_bias):
    in_maps = _prep_inputs(v, q, Wv, gv, bv, Wq, gq, bq, h_mat, h_bias)
    res = run_device(in_maps)
    out = np.empty((B, H, N, D), np.float32)
    for h in range(NCORES):
        out[:, h] = res.results[h]["out"].astype(np.float32)
    return out


if __name__ == "__main__":
    rng = np.random.default_rng(0)
    ins = {
        "v": rng.standard_normal((B, N, C), np.float32),
        "q": rng.standard_normal((B, D, C), np.float32),
        "Wv": rng.standard_normal((K, C), np.float32) * 0.02,
        "gv": np.ones((), np.float32),
        "bv": rng.standard_normal((K,), np.float32) * 0.02,
        "Wq": rng.standard_normal((K, C), np.float32) * 0.02,
        "gq": np.ones((), np.float32),
        "bq": rng.standard_normal((K,), np.float32) * 0.02,
        "h_mat": rng.standard_normal((1, H, 1, K), np.float32) * 0.02,
        "h_bias": np.zeros((1, H, 1, 1), np.float32),
    }
    out = kernel(**ins)
    print("out", out.shape, out.dtype, np.abs(out).max())


# revision 20
# speedup vs baseline: 10.3886x; 1.0058x over previous
"""Trainium2 Bass kernel for BCNet-style bilinear head.

Computes logits[b,h,n,d] = sum_k hm[h,k] * v_[b,n,k] * q_[b,d,k] + h_bias
where v_ = v @ wn(Wv,gv).T + bv,  q_ = q @ wn(Wq,gq).T + bq,
wn(W,g) = (g/||W||_F) * W.

Head-parallel M-route (120 GF total vs 150 GF for the GT-route):
expand the product; per head h (= per core):
  Mt[c',c]   = sum_k hm[h,k]*Wq'[k,c'] * Wv'[k,c]     (C x C, batch-indep)
  PT[c,bd]   = sum_c' Mt[c',c] * qT[c',bd] (+ u[c])   (u absorbs bq-term)
  out[b,n,d] = sum_c vT[b,c,n] * PT[c,b*D+d] + t3[b,d]
  t3[bd]     = sum_c' w[c'] * qT[c',bd] + t4          (bv-term + const)
with u[c] = sum_k hm*bq*Wv'[k,c], w[c'] = sum_k hm*bv*Wq'[k,c'],
t4 = sum_k hm*bv*bq + h_bias[h].
Sharding: head-parallel over H=8 across 8 cores; each core consumes the
full v/q (replicated) and emits out[:, h] — no collectives.
All matmuls bf16 with fp32 PSUM accumulation.

PSUM: one pool, 8 tags of [128,512] (16KB/part), reused by tag across
M / t3 / P / out phases. vT tiles ride a 48-slot ring over the retired
wqh/wv SBUF slots (M pass 2 walks kt in reverse so high-kt slots retire
first). P is software-pipelined one batch ahead of out to hide the
PSUM->SBUF copy latency.
"""

import sys

for _p in ("/opt/trn_rl_repo",):
    if _p not in sys.path:
        sys.path.insert(0, _p)

import numpy as np
import ml_dtypes

from concourse import bass, bacc, tile, mybir
from concourse.bass_utils import run_bass_kernel_spmd

BF16 = ml_dtypes.bfloat16
F32 = mybir.dt.float32
BF = mybir.dt.bfloat16
AF = mybir.ActivationFunctionType

B, N, C, D, K, H = 16, 1024, 1024, 128, 3072, 8
KT, CT, NT = K // 128, C // 128, N // 128  # 24, 8, 8
BD = B * D  # 2048
NCORES = 8
XU, XW, XT4 = 0, CT, 2 * CT  # cst columns: u tiles, w tiles, t4

_CACHE = {}


def _build_program(repeat=1):
    nc = bacc.Bacc("TRN2", target_bir_lowering=False, debug=False,
                   num_devices=NCORES)

    # wqh[kt,p,c'] = hm[h,k]*Wq'[k,c'], k = kt*128+p   (per-core, head h)
    wqh_d = nc.dram_tensor("wqh", [KT, 128, C], BF, kind="ExternalInput")
    wv_d = nc.dram_tensor("wv", [KT, 128, C], BF, kind="ExternalInput")
    # qT[ct,p,b*128+d] = q[b,d,ct*128+p]
    qT_d = nc.dram_tensor("qT", [CT, 128, BD], BF, kind="ExternalInput")
    # vT[b,ct,p,n] = v[b,n,ct*128+p]
    vT_d = nc.dram_tensor("vT", [B, CT, 128, N], BF, kind="ExternalInput")
    cst_d = nc.dram_tensor("cst", [128, 2 * CT + 1], F32, kind="ExternalInput")
    one_d = nc.dram_tensor("one", [128, 1], BF, kind="ExternalInput")
    oner_d = nc.dram_tensor("oner", [1, 128], BF, kind="ExternalInput")
    out_d = nc.dram_tensor("out", [B, N, D], BF, kind="ExternalOutput")

    with tile.TileContext(nc) as tc:
        with (
            tc.tile_pool(name="wq", bufs=1) as p_wq,
            tc.tile_pool(name="wv", bufs=1) as p_wv,
            tc.tile_pool(name="qt", bufs=1) as p_qt,
            tc.tile_pool(name="mt", bufs=1) as p_mt,
            tc.tile_pool(name="pt", bufs=1) as p_pt,
            tc.tile_pool(name="t3", bufs=1) as p_t3,
            tc.tile_pool(name="small", bufs=1) as p_small,
            tc.tile_pool(name="ob", bufs=1) as p_ob,
            tc.tile_pool(name="ps", bufs=1, space="PSUM") as ps,
        ):
          for rep in range(repeat):
            R = f"r{rep}_"
            # ---- DMA: kt=0 weights first so M starts immediately ----
            # kt=0 is split into half-tiles so the very first matmul only
            # waits on a 128KB DMA instead of 2x256KB
            wq_sb, wv_sb = [], []
            for kt in range(KT):
                if kt == 0:
                    tq = tuple(
                        p_wq.tile([128, 512], BF, tag=f"wq0{hb}",
                                  name=f"{R}wq0{hb}") for hb in "ab")
                    tv = tuple(
                        p_wv.tile([128, 512], BF, tag=f"wv0{hb}",
                                  name=f"{R}wv0{hb}") for hb in "ab")
                    nc.sync.dma_start(tq[0][:], wqh_d.ap()[0, :, 0:512])
                    nc.sync.dma_start(tv[0][:], wv_d.ap()[0, :, 0:512])
                    nc.sync.dma_start(tq[1][:], wqh_d.ap()[0, :, 512:1024])
                    nc.sync.dma_start(tv[1][:], wv_d.ap()[0, :, 512:1024])
                    wq_sb.append(tq)
                    wv_sb.append(tv)
                else:
                    tq = p_wq.tile([128, C], BF, tag=f"wq{kt}",
                                   name=f"{R}wq{kt}")
                    nc.sync.dma_start(tq[:], wqh_d.ap()[kt])
                    wq_sb.append(tq)
                    tv = p_wv.tile([128, C], BF, tag=f"wv{kt}",
                                   name=f"{R}wv{kt}")
                    nc.sync.dma_start(tv[:], wv_d.ap()[kt])
                    wv_sb.append(tv)
                if kt == 0:
                    # small consts ride the scalar engine's DGE queue
                    cst_sb = p_small.tile([128, 2 * CT + 1], F32, tag="cst",
                                          name=f"{R}cst")
                    nc.scalar.dma_start(cst_sb[:], cst_d.ap())
                    one_sb = p_small.tile([128, 1], BF, tag="one",
                                          name=f"{R}one")
                    nc.scalar.dma_start(one_sb[:], one_d.ap())
                    oner_sb = p_small.tile([1, 128], BF, tag="oner",
                                           name=f"{R}oner")
                    nc.scalar.dma_start(oner_sb[:], oner_d.ap())
                    qt_sb = p_qt.tile([128, CT * BD], BF, tag="qt",
                                      name=f"{R}qt")


            # qT streams on the sync queue after all weight tiles: M pass 1
            # consumes the weight stream at DMA rate, while pass 2 re-reads
            # SBUF — qT (and then vT) ride the pass-2 bus-idle window
            for g in range(CT):
                nc.sync.dma_start(qt_sb[:, g * BD:(g + 1) * BD], qT_d.ap()[g])

            # ---- t3 partials on DVE (runs during M) ----
            ta = p_t3.tile([128, BD], BF, tag="ta", name=f"{R}ta")
            tb = p_t3.tile([128, BD], BF, tag="tb", name=f"{R}tb")
            nc.vector.tensor_scalar_mul(ta[:], qt_sb[:, 0:BD],
                                        cst_sb[:, XW:XW + 1])
            for ct in range(1, CT):
                nc.vector.tensor_scalar_mul(
                    tb[:], qt_sb[:, ct * BD:(ct + 1) * BD],
                    cst_sb[:, XW + ct:XW + ct + 1])
                nc.vector.tensor_tensor(ta[:], ta[:], tb[:],
                                        mybir.AluOpType.add)

            # ---- M: Mt[c',c] = sum_k wqh[k,c']*wv[k,c] ----
            # two c-half passes; pass 2 reversed so high-kt tiles retire
            # first (their SBUF slots become the vT ring, below)
            mt_sb = [p_mt.tile([128, C], BF, tag=f"mt{i}", name=f"{R}mt{i}")
                     for i in range(CT)]
            for half in range(2):
                kts = list(range(KT)) if half == 0 else \
                    list(range(KT - 1, -1, -1))
                pms = [ps.tile([128, 512], F32, tag=f"t{i}",
                               name=f"{R}pm{half}_{i}") for i in range(CT)]
                for kt in kts:
                    for i in range(CT):
                        if kt == 0:
                            lhsT = wq_sb[0][i // 4][:, (i % 4) * 128:
                                                    (i % 4 + 1) * 128]
                            rhs = wv_sb[0][half][:]
                        else:
                            lhsT = wq_sb[kt][:, i * 128:(i + 1) * 128]
                            rhs = wv_sb[kt][:, half * 512:(half + 1) * 512]
                        nc.tensor.matmul(
                            pms[i][:], lhsT, rhs,
                            start=(kt == kts[0]), stop=(kt == kts[-1]))
                for i in range(CT):
                    dst = mt_sb[i][:, half * 512:(half + 1) * 512]
                    if i % 2 == 0:
                        nc.scalar.activation(dst, pms[i][:], AF.Copy)
                    else:
                        nc.vector.tensor_copy(dst, pms[i][:])

            # ---- t3 row: partition-reduce + t4; broadcast via k=1 matmul.
            # Issued on PE after P_0 (t3bc is first needed by out_0's adds),
            # so P_0 starts the moment M finishes.
            t3row = p_t3.tile([1, BD], BF, tag="t3row", name=f"{R}t3row")
            t3bc = p_t3.tile([128, BD], BF, tag="t3bc", name=f"{R}t3bc")

            def t3_phase():
                for j in range(4):
                    pt3 = ps.tile([1, 512], F32, tag=f"t{j}",
                                  name=f"{R}t3ps{j}")
                    nc.tensor.matmul(pt3[:], one_sb[:, 0:1],
                                     ta[:, j * 512:(j + 1) * 512],
                                     start=True, stop=True)
                    nc.scalar.activation(t3row[0:1, j * 512:(j + 1) * 512],
                                         pt3[:], AF.Identity,
                                         bias=cst_sb[0:1, XT4:XT4 + 1],
                                         scale=1.0)
                for j in range(4):
                    pb = ps.tile([128, 512], F32, tag=f"t{4 + j}",
                                 name=f"{R}t3bc{j}")
                    nc.tensor.matmul(pb[:], oner_sb[:],
                                     t3row[0:1, j * 512:(j + 1) * 512],
                                     start=True, stop=True)
                    if j % 2 == 0:
                        nc.scalar.activation(t3bc[:, j * 512:(j + 1) * 512],
                                             pb[:], AF.Copy)
                    else:
                        nc.vector.tensor_copy(
                            t3bc[:, j * 512:(j + 1) * 512], pb[:])

            # ---- per batch: P_b (pipelined one ahead) + out_{b-1} ----
            pt_sb = [p_pt.tile([128, BD], BF, tag=f"pt{i}", name=f"{R}pt{i}")
                     for i in range(CT)]
            vts = {}

            def load_vt(b):
                vts[b] = []
                for ct in range(CT):
                    # ring over kt 1..23 slots (kt=0 was split into halves)
                    g = (b * CT + ct) % (2 * (KT - 1))
                    skt = KT - 1 - g // 2
                    pool, tag = ((p_wq, f"wq{skt}") if g % 2 == 0
                                 else (p_wv, f"wv{skt}"))
                    t = pool.tile([128, C], BF, tag=tag, name=f"{R}vt{b}_{ct}")
                    nc.sync.dma_start(t[:], vT_d.ap()[b, ct])
                    vts[b].append(t)

            def p_phase(b):
                for ct in range(CT):
                    pp = ps.tile([128, 128], F32, tag=f"t{ct}",
                                 name=f"{R}pp{b}_{ct}")
                    for j in range(CT):
                        nc.tensor.matmul(
                            pp[:],
                            mt_sb[j][:, ct * 128:(ct + 1) * 128],
                            qt_sb[:, j * BD + b * 128:j * BD + (b + 1) * 128],
                            start=(j == 0), stop=(j == CT - 1))
                    nc.scalar.activation(
                        pt_sb[ct][:, b * 128:(b + 1) * 128], pp[:],
                        AF.Identity, bias=cst_sb[:, XU + ct:XU + ct + 1],
                        scale=1.0)

            def out_phase(b):
                ob = p_ob.tile([128, NT * D], BF, tag=f"ob{b % 2}",
                               name=f"{R}ob{b}")
                for nt in range(NT):
                    po = ps.tile([128, 128], F32, tag=f"t{nt}",
                                 name=f"{R}po{b}_{nt}")
                    for ct in range(CT):
                        nc.tensor.matmul(
                            po[:],
                            vts[b][ct][:, nt * 128:(nt + 1) * 128],
                            pt_sb[ct][:, b * 128:(b + 1) * 128],
                            start=(ct == 0), stop=(ct == CT - 1))
                    nc.vector.tensor_tensor(
                        ob[:, nt * D:(nt + 1) * D], po[:],
                        t3bc[:, b * 128:(b + 1) * 128],
                        mybir.AluOpType.add)
                # one store per batch, on Activation's DGE queue (doesn't
                # block the sync-queue vT load stream); the final batch is
                # split in two so the first half overlaps the last adds
                if b == B - 1:
                    for g in range(4):
                        nc.scalar.dma_start(
                            out_d.ap()[b, g * 256:(g + 1) * 256, :]
                            .rearrange("(nt p) d -> p nt d", p=128),
                            ob[:, g * 2 * D:(g + 1) * 2 * D]
                            .rearrange("p (nt d) -> p nt d", nt=2))
                else:
                    nc.scalar.dma_start(
                        out_d.ap()[b].rearrange("(nt p) d -> p nt d", p=128),
                        ob[:].rearrange("p (nt d) -> p nt d", nt=NT))
                del vts[b]

            load_vt(0)
            p_phase(0)
            t3_phase()
            for b in range(1, B):
                load_vt(b)
                p_phase(b)
                out_phase(b - 1)
            out_phase(B - 1)

    nc.compile()
    return nc


def _get_program(repeat=1):
    key = f"nc{repeat}"
    if key not in _CACHE:
        _CACHE[key] = _build_program(repeat)
    return _CACHE[key]


def _prep_inputs(v, q, Wv, gv, bv, Wq, gq, bq, h_mat, h_bias):
    v = np.asarray(v, np.float32)
    q = np.asarray(q, np.float32)
    Wv = np.asarray(Wv, np.float32)
    Wq = np.asarray(Wq, np.float32)
    bv = np.asarray(bv, np.float32)
    bq = np.asarray(bq, np.float32)
    sv = np.float32(gv) / np.float32(np.linalg.norm(Wv))
    sq = np.float32(gq) / np.float32(np.linalg.norm(Wq))
    hm = np.asarray(h_mat, np.float32)[0, :, 0, :]  # (H, K)
    hb = np.asarray(h_bias, np.float32).reshape(H)

    Wvp = Wv * sv  # (K, C)
    Wqp = Wq * sq
    wv_b = np.ascontiguousarray(Wvp.reshape(KT, 128, C)).astype(BF16)
    qT = np.ascontiguousarray(
        q.transpose(2, 0, 1).reshape(CT, 128, BD)).astype(BF16)
    vT = np.ascontiguousarray(
        v.transpose(0, 2, 1).reshape(B, CT, 128, N)).astype(BF16)
    one = np.ones((128, 1), BF16)
    oner = np.ones((1, 128), BF16)

    in_maps = []
    for h in range(NCORES):
        wqh = np.ascontiguousarray(
            (hm[h][:, None] * Wqp).reshape(KT, 128, C)).astype(BF16)
        u = (hm[h] * bq) @ Wvp  # (C,)
        w = (hm[h] * bv) @ Wqp  # (C,)
        t4 = float((hm[h] * bv) @ bq) + float(hb[h])
        cst = np.zeros((128, 2 * CT + 1), np.float32)
        cst[:, XU:XU + CT] = u.reshape(CT, 128).T
        cst[:, XW:XW + CT] = w.reshape(CT, 128).T
        cst[0, XT4] = t4
        in_maps.append({
            "wqh": wqh,
            "wv": wv_b,
            "qT": qT,
            "vT": vT,
            "cst": cst,
            "one": one,
            "oner": oner,
        })
    return in_maps


def run_device(in_maps, **kw):
    nc = _get_program()
    return run_bass_kernel_spmd(nc, in_maps, list(range(NCORES)), **kw)


def kernel(v, q, Wv, gv, bv, Wq, gq, bq, h_mat, h_bias):
    in_maps = _prep_inputs(v, q, Wv, gv, bv, Wq, gq, bq, h_mat, h_bias)
    res = run_device(in_maps)
    out = np.empty((B, H, N, D), np.float32)
    for h in range(NCORES):
        out[:, h] = res.results[h]["out"].astype(np.float32)
    return out


if __name__ == "__main__":
    rng = np.random.default_rng(0)
    ins = {
        "v": rng.standard_normal((B, N, C), np.float32),
        "q": rng.standard_normal((B, D, C), np.float32),
        "Wv": rng.standard_normal((K, C), np.float32) * 0.02,
        "gv": np.ones((), np.float32),
        "bv": rng.standard_normal((K,), np.float32) * 0.02,
        "Wq": rng.standard_normal((K, C), np.float32) * 0.02,
        "gq": np.ones((), np.float32),
        "bq": rng.standard_normal((K,), np.float32) * 0.02,
        "h_mat": rng.standard_normal((1, H, 1, K), np.float32) * 0.02,
        "h_bias": np.zeros((1, H, 1, 1), np.float32),
    }
    out = kernel(**ins)
    print("out", out.shape, out.dtype, np.abs(out).max())


# revision 27
# speedup vs baseline: 10.3946x; 1.0006x over previous
"""Trainium2 Bass kernel for BCNet-style bilinear head.

Computes logits[b,h,n,d] = sum_k hm[h,k] * v_[b,n,k] * q_[b,d,k] + h_bias
where v_ = v @ wn(Wv,gv).T + bv,  q_ = q @ wn(Wq,gq).T + bq,
wn(W,g) = (g/||W||_F) * W.

Head-parallel M-route (120 GF total vs 150 GF for the GT-route):
expand the product; per head h (= per core):
  Mt[c',c]   = sum_k hm[h,k]*Wq'[k,c'] * Wv'[k,c]     (C x C, batch-indep)
  PT[c,bd]   = sum_c' Mt[c',c] * qT[c',bd] (+ u[c])   (u absorbs bq-term)
  out[b,n,d] = sum_c vT[b,c,n] * PT[c,b*D+d] + t3[b,d]
  t3[bd]     = sum_c' w[c'] * qT[c',bd] + t4          (bv-term + const)
with u[c] = sum_k hm*bq*Wv'[k,c], w[c'] = sum_k hm*bv*Wq'[k,c'],
t4 = sum_k hm*bv*bq + h_bias[h].
Sharding: head-parallel over H=8 across 8 cores; each core consumes the
full v/q (replicated) and emits out[:, h] — no collectives.
All matmuls bf16 with fp32 PSUM accumulation.

PSUM: one pool, 8 tags of [128,512] (16KB/part), reused by tag across
M / t3 / P / out phases. vT tiles ride a 48-slot ring over the retired
wqh/wv SBUF slots (M pass 2 walks kt in reverse so high-kt slots retire
first). P is software-pipelined one batch ahead of out to hide the
PSUM->SBUF copy latency.
"""

import sys

for _p in ("/opt/trn_rl_repo",):
    if _p not in sys.path:
        sys.path.insert(0, _p)

import numpy as np
import ml_dtypes

from concourse import bass, bacc, tile, mybir
from concourse.bass_utils import run_bass_kernel_spmd

BF16 = ml_dtypes.bfloat16
F32 = mybir.dt.float32
BF = mybir.dt.bfloat16
AF = mybir.ActivationFunctionType

B, N, C, D, K, H = 16, 1024, 1024, 128, 3072, 8
KT, CT, NT = K // 128, C // 128, N // 128  # 24, 8, 8
BD = B * D  # 2048
NCORES = 8
XU, XW, XT4 = 0, CT, 2 * CT  # cst columns: u tiles, w tiles, t4

_CACHE = {}


def _build_program(repeat=1):
    nc = bacc.Bacc("TRN2", target_bir_lowering=False, debug=False,
                   num_devices=NCORES)

    # wqh[kt,p,c'] = hm[h,k]*Wq'[k,c'], k = kt*128+p   (per-core, head h)
    wqh_d = nc.dram_tensor("wqh", [KT, 128, C], BF, kind="ExternalInput")
    wv_d = nc.dram_tensor("wv", [KT, 128, C], BF, kind="ExternalInput")
    # qT[ct,p,b*128+d] = q[b,d,ct*128+p]
    qT_d = nc.dram_tensor("qT", [CT, 128, BD], BF, kind="ExternalInput")
    # vT[b,ct,p,n] = v[b,n,ct*128+p]
    vT_d = nc.dram_tensor("vT", [B, CT, 128, N], BF, kind="ExternalInput")
    cst_d = nc.dram_tensor("cst", [128, 2 * CT + 1], F32, kind="ExternalInput")
    one_d = nc.dram_tensor("one", [128, 1], BF, kind="ExternalInput")
    oner_d = nc.dram_tensor("oner", [1, 128], BF, kind="ExternalInput")
    out_d = nc.dram_tensor("out", [B, N, D], BF, kind="ExternalOutput")

    with tile.TileContext(nc) as tc:
        with (
            tc.tile_pool(name="wq", bufs=1) as p_wq,
            tc.tile_pool(name="wv", bufs=1) as p_wv,
            tc.tile_pool(name="qt", bufs=1) as p_qt,
            tc.tile_pool(name="mt", bufs=1) as p_mt,
            tc.tile_pool(name="pt", bufs=1) as p_pt,
            tc.tile_pool(name="t3", bufs=1) as p_t3,
            tc.tile_pool(name="small", bufs=1) as p_small,
            tc.tile_pool(name="ob", bufs=1) as p_ob,
            tc.tile_pool(name="vt", bufs=1) as p_vt,
            tc.tile_pool(name="ps", bufs=1, space="PSUM") as ps,
        ):
          for rep in range(repeat):
            R = f"r{rep}_"
            # ---- DMA: kt=0 weights first so M starts immediately ----
            # M pass 1 consumes wq[kt] + the low c-half of wv[kt] at
            # 1.7us/kt; streaming only those (1.09us/kt) keeps the PE fed.
            # wv high halves follow afterward (pass 2's window). kt=0's wq
            # is further split so the first matmul waits on a 128KB DMA.
            wq_sb, wv_sb = [], []
            for kt in range(KT):
                tv = [p_wv.tile([128, 512], BF, tag=f"wv{kt}{hb}",
                                name=f"{R}wv{kt}{hb}") for hb in "ab"]
                wv_sb.append(tv)
                if kt == 0:
                    tq = tuple(
                        p_wq.tile([128, 512], BF, tag=f"wq0{hb}",
                                  name=f"{R}wq0{hb}") for hb in "ab")
                    nc.sync.dma_start(tq[0][:], wqh_d.ap()[0, :, 0:512])
                    nc.sync.dma_start(tv[0][:], wv_d.ap()[0, :, 0:512])
                    nc.sync.dma_start(tq[1][:], wqh_d.ap()[0, :, 512:1024])
                    wq_sb.append(tq)
                else:
                    tq = p_wq.tile([128, C], BF, tag=f"wq{kt}",
                                   name=f"{R}wq{kt}")
                    nc.sync.dma_start(tq[:], wqh_d.ap()[kt])
                    wq_sb.append(tq)
                    nc.sync.dma_start(tv[0][:], wv_d.ap()[kt, :, 0:512])
                if kt == 0:
                    # small consts ride the scalar engine's DGE queue
                    cst_sb = p_small.tile([128, 2 * CT + 1], F32, tag="cst",
                                          name=f"{R}cst")
                    nc.scalar.dma_start(cst_sb[:], cst_d.ap())
                    one_sb = p_small.tile([128, 1], BF, tag="one",
                                          name=f"{R}one")
                    nc.scalar.dma_start(one_sb[:], one_d.ap())
                    oner_sb = p_small.tile([1, 128], BF, tag="oner",
                                           name=f"{R}oner")
                    nc.scalar.dma_start(oner_sb[:], oner_d.ap())
                    qt_sb = p_qt.tile([128, CT * BD], BF, tag="qt",
                                      name=f"{R}qt")


            # wv high halves (pass 2 runs kt reversed, so send them
            # reversed), then qT, then vT — all riding the bus window left
            # idle once the pass-1 stream ends
            for kt in range(KT - 1, -1, -1):
                nc.sync.dma_start(wv_sb[kt][1][:],
                                  wv_d.ap()[kt, :, 512:1024])
            for g in range(CT):
                nc.sync.dma_start(qt_sb[:, g * BD:(g + 1) * BD], qT_d.ap()[g])

            # ---- t3 partials on DVE (runs during M) ----
            ta = p_t3.tile([128, BD], BF, tag="ta", name=f"{R}ta")
            tb = p_t3.tile([128, BD], BF, tag="tb", name=f"{R}tb")
            nc.vector.tensor_scalar_mul(ta[:], qt_sb[:, 0:BD],
                                        cst_sb[:, XW:XW + 1])
            for ct in range(1, CT):
                nc.vector.tensor_scalar_mul(
                    tb[:], qt_sb[:, ct * BD:(ct + 1) * BD],
                    cst_sb[:, XW + ct:XW + ct + 1])
                nc.vector.tensor_tensor(ta[:], ta[:], tb[:],
                                        mybir.AluOpType.add)

            # ---- M: Mt[c',c] = sum_k wqh[k,c']*wv[k,c] ----
            # two c-half passes; pass 2 reversed so high-kt tiles retire
            # first (their SBUF slots become the vT ring, below)
            mt_sb = [p_mt.tile([128, C], BF, tag=f"mt{i}", name=f"{R}mt{i}")
                     for i in range(CT)]
            for half in range(2):
                kts = list(range(KT)) if half == 0 else \
                    list(range(KT - 1, -1, -1))
                pms = [ps.tile([128, 512], F32, tag=f"t{i}",
                               name=f"{R}pm{half}_{i}") for i in range(CT)]
                for kt in kts:
                    for i in range(CT):
                        if kt == 0:
                            lhsT = wq_sb[0][i // 4][:, (i % 4) * 128:
                                                    (i % 4 + 1) * 128]
                        else:
                            lhsT = wq_sb[kt][:, i * 128:(i + 1) * 128]
                        nc.tensor.matmul(
                            pms[i][:], lhsT, wv_sb[kt][half][:],
                            start=(kt == kts[0]), stop=(kt == kts[-1]))
                for i in range(CT):
                    dst = mt_sb[i][:, half * 512:(half + 1) * 512]
                    if i % 2 == 0:
                        nc.scalar.activation(dst, pms[i][:], AF.Copy)
                    else:
                        nc.vector.tensor_copy(dst, pms[i][:])

            # ---- t3 row: partition-reduce + t4; broadcast via k=1 matmul.
            # Issued on PE after P_0 (t3bc is first needed by out_0's adds),
            # so P_0 starts the moment M finishes.
            t3row = p_t3.tile([1, BD], BF, tag="t3row", name=f"{R}t3row")
            t3bc = p_t3.tile([128, BD], BF, tag="t3bc", name=f"{R}t3bc")

            def t3_phase():
                for j in range(4):
                    pt3 = ps.tile([1, 512], F32, tag=f"t{j}",
                                  name=f"{R}t3ps{j}")
                    nc.tensor.matmul(pt3[:], one_sb[:, 0:1],
                                     ta[:, j * 512:(j + 1) * 512],
                                     start=True, stop=True)
                    nc.scalar.activation(t3row[0:1, j * 512:(j + 1) * 512],
                                         pt3[:], AF.Identity,
                                         bias=cst_sb[0:1, XT4:XT4 + 1],
                                         scale=1.0)
                for j in range(4):
                    pb = ps.tile([128, 512], F32, tag=f"t{4 + j}",
                                 name=f"{R}t3bc{j}")
                    nc.tensor.matmul(pb[:], oner_sb[:],
                                     t3row[0:1, j * 512:(j + 1) * 512],
                                     start=True, stop=True)
                    if j % 2 == 0:
                        nc.scalar.activation(t3bc[:, j * 512:(j + 1) * 512],
                                             pb[:], AF.Copy)
                    else:
                        nc.vector.tensor_copy(
                            t3bc[:, j * 512:(j + 1) * 512], pb[:])

            # ---- per batch: P_b (pipelined one ahead) + out_{b-1} ----
            pt_sb = [p_pt.tile([128, BD], BF, tag=f"pt{i}", name=f"{R}pt{i}")
                     for i in range(CT)]
            vts = {}

            # vT ring: 7 dedicated fresh slots (usable before M retires
            # anything), then the 23 full wq slots in pass-2 retire order
            NVP = 5
            RING = NVP + (KT - 1)

            def load_vt(b):
                vts[b] = []
                for ct in range(CT):
                    g = (b * CT + ct) % RING
                    if g < NVP:
                        pool, tag = p_vt, f"vtp{g}"
                    else:
                        pool, tag = p_wq, f"wq{KT - 1 - (g - NVP)}"
                    t = pool.tile([128, C], BF, tag=tag, name=f"{R}vt{b}_{ct}")
                    nc.sync.dma_start(t[:], vT_d.ap()[b, ct])
                    vts[b].append(t)

            def p_phase(b):
                for ct in range(CT):
                    pp = ps.tile([128, 128], F32, tag=f"t{ct}",
                                 name=f"{R}pp{b}_{ct}")
                    for j in range(CT):
                        nc.tensor.matmul(
                            pp[:],
                            mt_sb[j][:, ct * 128:(ct + 1) * 128],
                            qt_sb[:, j * BD + b * 128:j * BD + (b + 1) * 128],
                            start=(j == 0), stop=(j == CT - 1))
                    nc.scalar.activation(
                        pt_sb[ct][:, b * 128:(b + 1) * 128], pp[:],
                        AF.Identity, bias=cst_sb[:, XU + ct:XU + ct + 1],
                        scale=1.0)

            def out_phase(b):
                ob = p_ob.tile([128, NT * D], BF, tag=f"ob{b % 2}",
                               name=f"{R}ob{b}")
                for nt in range(NT):
                    po = ps.tile([128, 128], F32, tag=f"t{nt}",
                                 name=f"{R}po{b}_{nt}")
                    for ct in range(CT):
                        nc.tensor.matmul(
                            po[:],
                            vts[b][ct][:, nt * 128:(nt + 1) * 128],
                            pt_sb[ct][:, b * 128:(b + 1) * 128],
                            start=(ct == 0), stop=(ct == CT - 1))
                    nc.vector.tensor_tensor(
                        ob[:, nt * D:(nt + 1) * D], po[:],
                        t3bc[:, b * 128:(b + 1) * 128],
                        mybir.AluOpType.add)
                # one store per batch, on Activation's DGE queue (doesn't
                # block the sync-queue vT load stream); the final batch is
                # split in two so the first half overlaps the last adds
                if b == B - 1:
                    for g in range(4):
                        eng = nc.scalar if g % 2 == 0 else nc.sync
                        eng.dma_start(
                            out_d.ap()[b, g * 256:(g + 1) * 256, :]
                            .rearrange("(nt p) d -> p nt d", p=128),
                            ob[:, g * 2 * D:(g + 1) * 2 * D]
                            .rearrange("p (nt d) -> p nt d", nt=2))
                else:
                    nc.scalar.dma_start(
                        out_d.ap()[b].rearrange("(nt p) d -> p nt d", p=128),
                        ob[:].rearrange("p (nt d) -> p nt d", nt=NT))
                del vts[b]

            load_vt(0)
            p_phase(0)
            t3_phase()
            for b in range(1, B):
                load_vt(b)
                p_phase(b)
                out_phase(b - 1)
            out_phase(B - 1)

    nc.compile()
    return nc


def _get_program(repeat=1):
    key = f"nc{repeat}"
    if key not in _CACHE:
        _CACHE[key] = _build_program(repeat)
    return _CACHE[key]


def _prep_inputs(v, q, Wv, gv, bv, Wq, gq, bq, h_mat, h_bias):
    v = np.asarray(v, np.float32)
    q = np.asarray(q, np.float32)
    Wv = np.asarray(Wv, np.float32)
    Wq = np.asarray(Wq, np.float32)
    bv = np.asarray(bv, np.float32)
    bq = np.asarray(bq, np.float32)
    sv = np.float32(gv) / np.float32(np.linalg.norm(Wv))
    sq = np.float32(gq) / np.float32(np.linalg.norm(Wq))
    hm = np.asarray(h_mat, np.float32)[0, :, 0, :]  # (H, K)
    hb = np.asarray(h_bias, np.float32).reshape(H)

    Wvp = Wv * sv  # (K, C)
    Wqp = Wq * sq
    wv_b = np.ascontiguousarray(Wvp.reshape(KT, 128, C)).astype(BF16)
    qT = np.ascontiguousarray(
        q.transpose(2, 0, 1).reshape(CT, 128, BD)).astype(BF16)
    vT = np.ascontiguousarray(
        v.transpose(0, 2, 1).reshape(B, CT, 128, N)).astype(BF16)
    one = np.ones((128, 1), BF16)
    oner = np.ones((1, 128), BF16)

    in_maps = []
    for h in range(NCORES):
        wqh = np.ascontiguousarray(
            (hm[h][:, None] * Wqp).reshape(KT, 128, C)).astype(BF16)
        u = (hm[h] * bq) @ Wvp  # (C,)
        w = (hm[h] * bv) @ Wqp  # (C,)
        t4 = float((hm[h] * bv) @ bq) + float(hb[h])
        cst = np.zeros((128, 2 * CT + 1), np.float32)
        cst[:, XU:XU + CT] = u.reshape(CT, 128).T
        cst[:, XW:XW + CT] = w.reshape(CT, 128).T
        cst[0, XT4] = t4
        in_maps.append({
            "wqh": wqh,
            "wv": wv_b,
            "qT": qT,
            "vT": vT,
            "cst": cst,
            "one": one,
            "oner": oner,
        })
    return in_maps


def run_device(in_maps, **kw):
    nc = _get_program()
    return run_bass_kernel_spmd(nc, in_maps, list(range(NCORES)), **kw)


def kernel(v, q, Wv, gv, bv, Wq, gq, bq, h_mat, h_bias):
    in_maps = _prep_inputs(v, q, Wv, gv, bv, Wq, gq, bq, h_mat, h_bias)
    res = run_device(in_maps)
    out = np.empty((B, H, N, D), np.float32)
    for h in range(NCORES):
        out[:, h] = res.results[h]["out"].astype(np.float32)
    return out


if __name__ == "__main__":
    rng = np.random.default_rng(0)
    ins = {
        "v": rng.standard_normal((B, N, C), np.float32),
        "q": rng.standard_normal((B, D, C), np.float32),
        "Wv": rng.standard_normal((K, C), np.float32) * 0.02,
        "gv": np.ones((), np.float32),
        "bv": rng.standard_normal((K,), np.float32) * 0.02,
        "Wq": rng.standard_normal((K, C), np.float32) * 0.02,
        "gq": np.ones((), np.float32),
        "bq": rng.standard_normal((K,), np.float32) * 0.02,
        "h_mat": rng.standard_normal((1, H, 1, K), np.float32) * 0.02,
        "h_bias": np.zeros((1, H, 1, 1), np.float32),
    }
    out = kernel(**ins)
    print("out", out.shape, out.dtype, np.abs(out).max())
